# revision 1
# baseline (speedup 1.0000x reference)
"""MultiHeadCrossAttention kernel for 8 Trainium2 NeuronCores.

Reference computation (b=2, nq=nk=2048, d_model=512, h=8, hd=64):
    Q = split_heads(q @ Wq.T + bq); K, V likewise
    S = Q K^T * hd^-0.5 ; A = softmax(S, -1) * mask_head * diag(pearson)[k]
    out = merge_heads(A @ V)

Sharding: 16 (batch, head) pairs -> 2 heads of one batch per core.
Only the *diagonal* of pearson_matrix is used, so it is extracted on the
host (128 KiB instead of 128 MiB of device traffic) and folded into the
mask, which is also transposed on the host so the device kernel can work
entirely in a "k on partitions, q on free axis" layout:

    S^T[k,q]   = sum_d K^T[d,k] Q^T[d,q]           (TensorE, d=64 contraction)
    E^T        = exp(SCALING * S^T)                (ScalarE, PSUM->SBUF)
    Z[q]       = sum_k E^T[k,q]                    (TensorE, ones-vector lhsT)
    A^T        = E^T * maskT_folded                (VectorE, mask streamed from HBM)
    agg^T[e,q] = sum_k V[k,e] A^T[k,q]             (TensorE, accumulated over k tiles)
    out^T      = agg^T / Z                         (VectorE, Z partition-broadcast)

The device returns out^T (128 rows = 2 heads x 64 dims, 2048 cols = q);
the host transposes and concatenates the 8 per-core slices.
"""

import ctypes
import os
import sys
import types

import numpy as np

import concourse.bacc as bacc
import concourse.bass as bass
import concourse.tile as tile
from concourse import mybir
from concourse.vector_clock import ScopedClock

F32 = mybir.dt.float32

B = 2
H = 8
N = 2048  # nq == nk
D = 512
HD = 64
HPC = 2  # heads per core
E = HPC * HD  # 128 output dims per core
SCALING = HD ** (-0.5)
NCORES = 8
P = 128
QC = 1024  # q super-chunk (2 per core)
NQC = N // QC
NKT = N // P  # 16 k tiles


# ---------------------------------------------------------------------------
# Page faults are extremely slow in this sandbox (~ms each); MAP_POPULATE
# prefaults an allocation in one syscall, ~100x faster for big arrays.
# ---------------------------------------------------------------------------
_libc = ctypes.CDLL(None, use_errno=True)
_libc.mmap.restype = ctypes.c_void_p
_libc.mmap.argtypes = [
    ctypes.c_void_p,
    ctypes.c_size_t,
    ctypes.c_int,
    ctypes.c_int,
    ctypes.c_int,
    ctypes.c_long,
]


def _alloc(shape, dtype=np.float32):
    nbytes = int(np.prod(shape)) * np.dtype(dtype).itemsize
    nbytes = (nbytes + 4095) & ~4095
    p = _libc.mmap(None, nbytes, 0x3, 0x02 | 0x20 | 0x8000, -1, 0)  # RW, PRIV|ANON|POPULATE
    if p in (None, ctypes.c_void_p(-1).value):
        return np.empty(shape, dtype)
    buf = (ctypes.c_byte * nbytes).from_address(p)
    return np.frombuffer(buf, dtype=dtype, count=int(np.prod(shape))).reshape(shape)


def _tcopy(src):
    """Contiguous transposed copy of a 2-D array into prefaulted memory."""
    dst = _alloc((src.shape[1], src.shape[0]), src.dtype)
    np.copyto(dst, src.T)
    return dst


# ---------------------------------------------------------------------------
# Environment shim: walrus in this container rejects >1 sync wait on
# CTRL-class instructions (NoOp/Drain), but TileContext's kernel-tail drain
# carries one wait per live semaphore.  Re-emit them as individual wait_ge
# instructions (one wait each) before a bare drain.
# ---------------------------------------------------------------------------
def _drain_and_barrier(self, tick_clock, wait_clock):
    probe = mybir.InstNoOp(
        name="wait_probe", ins=[], outs=[], engine=mybir.EngineType.SP
    )
    wait_clock.add_sem_waits(probe, ScopedClock({None: tick_clock.global_clock}))
    waits = list(probe.sync_info.on_wait) if probe.sync_info else []
    allocated = self.sems.allocated()
    by_name = {}
    for k, h in allocated.items():
        by_name[getattr(h, "name", str(k))] = h
    for w in waits:
        h = by_name.get(w.ant_name)
        assert h is not None, (w.ant_name, sorted(by_name))
        self.nc.sync.wait_ge(h, w.wait_value)
    self.nc.sync.drain()
    self.nc.all_engine_barrier()
    popped = self.nc._tile_sem_poison_stack.pop()
    assert popped is self._sem_poison
    self.nc.clear_and_free_semaphores(list(allocated.values()))
    self.nc.all_engine_barrier()


def _install_shims():
    tile.TileContext._drain_and_barrier = _drain_and_barrier
    if "antenv.axon_hooks" not in sys.modules:
        try:
            from trn_agent_boot.trn_boot import _ntff_profile_via_ctypes

            mod = types.ModuleType("antenv.axon_hooks")
            hook = _ntff_profile_via_ctypes("/opt/axon/libaxon_pjrt.so")
            mod.get_axon_ntff_profile_hook = lambda: hook
            mod.set_axon_ntff_profile_hook = lambda h: None
            sys.modules["antenv.axon_hooks"] = mod
        except Exception:
            pass


# ---------------------------------------------------------------------------
# Device kernel (one Bass program, SPMD over 8 cores; shards via in_maps)
# ---------------------------------------------------------------------------
def build_nc() -> bass.Bass:
    # KERNEL_F32R: 0 = f32 everywhere; 1 = AV matmul in float32r;
    # 2 = also S^T in float32r.
    f32r_level = int(os.environ.get("KERNEL_F32R", "0"))
    R = mybir.dt.float32r
    AT_DT = R if f32r_level >= 1 else F32
    QK_DT = R if f32r_level >= 2 else F32
    HF = 512  # matmul half-width (one PSUM bank)
    NHF = QC // HF

    nc = bacc.Bacc("TRN2", target_bir_lowering=False, debug=False)

    PRJ_DT = R if f32r_level >= 2 else F32
    qT = nc.dram_tensor("qT", [D, N], PRJ_DT, kind="ExternalInput")
    kT = nc.dram_tensor("kT", [D, N], PRJ_DT, kind="ExternalInput")
    vT = nc.dram_tensor("vT", [D, N], PRJ_DT, kind="ExternalInput")
    wqT = nc.dram_tensor("wqT", [D, E], PRJ_DT, kind="ExternalInput")
    wkT = nc.dram_tensor("wkT", [D, E], PRJ_DT, kind="ExternalInput")
    wvT = nc.dram_tensor("wvT", [D, E], PRJ_DT, kind="ExternalInput")
    bq = nc.dram_tensor("bq", [E, 1], F32, kind="ExternalInput")
    bk = nc.dram_tensor("bk", [E, 1], F32, kind="ExternalInput")
    bv = nc.dram_tensor("bv", [1, E], F32, kind="ExternalInput")
    # maskT[lh, k, q] = mask[b, h0+lh, q, k] * diag(pearson)[b, h0+lh, k]
    maskT = nc.dram_tensor("maskT", [HPC, N, N], F32, kind="ExternalInput")
    outT = nc.dram_tensor("outT", [E, N], F32, kind="ExternalOutput")
    # softmax denominators, normalization happens on the host
    zout = nc.dram_tensor("zout", [HPC, N], F32, kind="ExternalOutput")

    ncc = D // P  # 4 contraction chunks for the projections

    with tile.TileContext(nc) as tc:
        with (
            tc.tile_pool(name="consts", bufs=1) as consts,
            tc.tile_pool(name="persist", bufs=1) as persist,
            tc.tile_pool(name="ps_st", bufs=4, space="PSUM") as ps_st,
            tc.tile_pool(name="ps_agg", bufs=1, space="PSUM") as ps_agg,
        ):
            ones = consts.tile([P, 1], F32)
            nc.vector.memset(ones, 1.0)

            wq_sb = consts.tile([P, ncc, E], PRJ_DT, tag="wq")
            wk_sb = consts.tile([P, ncc, E], PRJ_DT, tag="wk")
            wv_sb = consts.tile([P, ncc, E], PRJ_DT, tag="wv")
            nc.sync.dma_start(out=wq_sb, in_=wqT[:, :].rearrange("(c p) e -> p c e", p=P))
            nc.sync.dma_start(out=wk_sb, in_=wkT[:, :].rearrange("(c p) e -> p c e", p=P))
            nc.sync.dma_start(out=wv_sb, in_=wvT[:, :].rearrange("(c p) e -> p c e", p=P))
            bq_sb = consts.tile([E, 1], F32, tag="bq")
            bk_sb = consts.tile([E, 1], F32, tag="bk")
            nc.sync.dma_start(out=bq_sb, in_=bq[:, :])
            nc.sync.dma_start(out=bk_sb, in_=bk[:, :])
            bv_sb = consts.tile([P, E], F32, tag="bv")
            nc.sync.dma_start(out=bv_sb, in_=bv[:, :].to_broadcast((P, E)))

            QT_sb = persist.tile([E, N], QK_DT, tag="QT")  # [e, n] 2 heads x 64
            KT_sb = persist.tile([E, N], QK_DT, tag="KT")
            V_sb = persist.tile([P, NKT, E], AT_DT, tag="V")  # [k%128, kt, e]

            # ---- phase 0: projections --------------------------------------
            with tc.tile_pool(name="qkv", bufs=1) as qkv:
                qts = [qkv.tile([P, N], PRJ_DT, name=f"q{c}", tag=f"q{c}") for c in range(ncc)]
                kts = [qkv.tile([P, N], PRJ_DT, name=f"k{c}", tag=f"k{c}") for c in range(ncc)]
                vts = [qkv.tile([P, N], PRJ_DT, name=f"v{c}", tag=f"v{c}") for c in range(ncc)]
                for c in range(ncc):
                    sl = slice(c * P, (c + 1) * P)
                    nc.sync.dma_start(out=qts[c], in_=qT[sl, :])
                    nc.sync.dma_start(out=kts[c], in_=kT[sl, :])
                    nc.sync.dma_start(out=vts[c], in_=vT[sl, :])

                # Q^T and K^T: [e, n] = sum_c w[c, e] * xT[c, n]
                for dst, w_sb, b_sb, srcs in (
                    (QT_sb, wq_sb, bq_sb, qts),
                    (KT_sb, wk_sb, bk_sb, kts),
                ):
                    for nch in range(N // HF):
                        cols = slice(nch * HF, (nch + 1) * HF)
                        ps = ps_st.tile([P, HF], F32, tag="st", name=f"psp{nch}")
                        for c in range(ncc):
                            nc.tensor.matmul(
                                ps,
                                w_sb[:, c, :],
                                srcs[c][:, cols],
                                start=(c == 0),
                                stop=(c == ncc - 1),
                            )
                        nc.scalar.activation(
                            dst[:, cols],
                            ps,
                            mybir.ActivationFunctionType.Identity,
                            bias=b_sb,
                        )

                # V natural: [n, e] = sum_c vT[c, n] * w[c, e]
                for t in range(NKT):
                    ps = ps_st.tile([P, HF], F32, tag="st", name=f"psv{t}")
                    for c in range(ncc):
                        nc.tensor.matmul(
                            ps[:, :E],
                            vts[c][:, t * P : (t + 1) * P],
                            wv_sb[:, c, :],
                            start=(c == 0),
                            stop=(c == ncc - 1),
                        )
                    nc.vector.tensor_add(V_sb[:, t, :], ps[:, :E], bv_sb)

            # ---- phase 1: attention (software-pipelined emission) ---------
            with (
                tc.tile_pool(name="et", bufs=2) as etp,
                tc.tile_pool(name="at", bufs=2) as atp,
                tc.tile_pool(name="eacc", bufs=2) as eaccp,
                tc.tile_pool(name="mask", bufs=4) as maskp,
                tc.tile_pool(name="small", bufs=2) as smallp,
                tc.tile_pool(name="outp", bufs=2) as outp,
            ):

                def emit_st(qc, kt, eaccs):
                    """S^T + exp + mask-mult + E-sum accumulate for one k tile."""
                    mt = maskp.tile(
                        [P, HPC, QC], F32, tag="mt", name=f"mt_{qc}_{kt}"
                    )
                    mask_ap = bass.AP(
                        tensor=maskT,
                        offset=kt * P * N + qc * QC,
                        ap=[[N, P], [N * N, HPC], [1, QC]],
                    )
                    nc.sync.dma_start(out=mt, in_=mask_ap)
                    tiles = []
                    kcols = slice(kt * P, (kt + 1) * P)
                    for half in range(NHF):
                        for lh in range(HPC):
                            hsl = slice(lh * HD, (lh + 1) * HD)
                            rcols = slice(
                                qc * QC + half * HF, qc * QC + (half + 1) * HF
                            )
                            st = ps_st.tile(
                                [P, HF], F32, tag="st", name=f"st_{qc}_{kt}_{lh}_{half}"
                            )
                            nc.tensor.matmul(
                                st,
                                KT_sb[hsl, kcols],
                                QT_sb[hsl, rcols],
                                start=True,
                                stop=True,
                                tile_position=(lh * HD, 0),
                            )
                            et = etp.tile(
                                [P, HF], F32, tag=f"et{lh}{half}",
                                name=f"et_{qc}_{kt}_{lh}_{half}",
                            )
                            nc.scalar.activation(
                                et, st, mybir.ActivationFunctionType.Exp, scale=SCALING
                            )
                            ea = eaccs[lh * NHF + half]
                            if kt == 0:
                                nc.vector.tensor_copy(ea, et)
                            else:
                                nc.vector.tensor_add(ea, ea, et)
                            at = atp.tile(
                                [P, HF], AT_DT, tag=f"at{lh}{half}",
                                name=f"at_{qc}_{kt}_{lh}_{half}",
                            )
                            nc.vector.tensor_mul(
                                at, et, mt[:, lh, half * HF : (half + 1) * HF]
                            )
                            tiles.append((lh, half, at))
                    return tiles

                def emit_av(kt, tiles, aggs):
                    first, last = kt == 0, kt == NKT - 1
                    for lh, half, at in tiles:
                        esl = slice(lh * HD, (lh + 1) * HD)
                        hcols = slice(half * HF, (half + 1) * HF)
                        nc.tensor.matmul(
                            aggs[lh][:, hcols],
                            V_sb[:, kt, esl],
                            at,
                            start=first,
                            stop=last,
                            skip_group_check=True,
                        )

                for qc in range(NQC):
                    qcols = slice(qc * QC, (qc + 1) * QC)
                    aggs = [
                        ps_agg.tile([HD, QC], F32, tag=f"agg{lh}", name=f"agg_{qc}_{lh}")
                        for lh in range(HPC)
                    ]
                    eaccs = [
                        eaccp.tile(
                            [P, HF], F32, tag=f"ea{i}", name=f"ea_{qc}_{i}"
                        )
                        for i in range(HPC * NHF)
                    ]
                    tiles = emit_st(qc, 0, eaccs)
                    for kt in range(1, NKT):
                        nxt = emit_st(qc, kt, eaccs)
                        emit_av(kt - 1, tiles, aggs)
                        tiles = nxt
                    emit_av(NKT - 1, tiles, aggs)

                    # Z = ones^T @ E_acc (partition-direction sum), one matmul
                    # per (head, half); head lh's row lands at partition 32*lh.
                    zsb = smallp.tile([33, QC], F32, tag="zsb", name=f"zsb{qc}")
                    for lh in range(HPC):
                        zrow = lh * 32
                        for half in range(NHF):
                            zp = ps_st.tile(
                                [33, HF], F32, tag="st", name=f"zp_{qc}_{lh}_{half}"
                            )
                            nc.tensor.matmul(
                                zp[zrow : zrow + 1, :],
                                ones,
                                eaccs[lh * NHF + half],
                                start=True,
                                stop=True,
                                tile_position=(0, zrow),
                            )
                            nc.vector.tensor_copy(
                                zsb[zrow : zrow + 1, half * HF : (half + 1) * HF],
                                zp[zrow : zrow + 1, :],
                            )
                        nc.sync.dma_start(
                            out=zout[lh, qcols], in_=zsb[zrow : zrow + 1, :]
                        )
                    for lh in range(HPC):
                        osb = outp.tile(
                            [HD, QC], F32, tag=f"osb{lh}", name=f"osb_{qc}_{lh}"
                        )
                        nc.vector.tensor_copy(osb, aggs[lh])
                        nc.sync.dma_start(
                            out=outT[lh * HD : (lh + 1) * HD, qcols], in_=osb
                        )

    nc.compile()
    return nc


# ---------------------------------------------------------------------------
# Host side
# ---------------------------------------------------------------------------
def _prep_in_maps(q, k, v, mask_head, pearson_matrix, Wq, bq, Wk, bk, Wv, bv):
    f = np.float32
    q = np.asarray(q, f)
    k = np.asarray(k, f)
    v = np.asarray(v, f)
    mask_head = np.asarray(mask_head, f)
    Wq = np.asarray(Wq, f)
    Wk = np.asarray(Wk, f)
    Wv = np.asarray(Wv, f)
    bq = np.asarray(bq, f)
    bk = np.asarray(bk, f)
    bv = np.asarray(bv, f)

    # Only the diagonal of pearson is used by the computation.
    pm = np.asarray(pearson_matrix)
    diag = np.ascontiguousarray(np.diagonal(pm, axis1=-2, axis2=-1)).astype(f)

    qT = [_tcopy(q[b]) for b in range(B)]
    kTt = [_tcopy(k[b]) for b in range(B)]
    vTt = [_tcopy(v[b]) for b in range(B)]

    # maskT_all[b, h, k, q] = mask[b, h, q, k] * diag[b, h, k]; per-core masks
    # are contiguous zero-copy views maskT_all[b, h0:h0+HPC].
    maskT_all = _alloc((B, H, N, N), f)
    for b in range(B):
        for h in range(H):
            np.multiply(mask_head[b, h].T, diag[b, h][:, None], out=maskT_all[b, h])

    in_maps = []
    for c in range(NCORES):
        b = c // (NCORES // B)
        h0 = HPC * (c % (NCORES // B))
        esl = slice(h0 * HD, (h0 + HPC) * HD)
        in_maps.append(
            {
                "qT": qT[b],
                "kT": kTt[b],
                "vT": vTt[b],
                "wqT": _tcopy(Wq[esl, :]),
                "wkT": _tcopy(Wk[esl, :]),
                "wvT": _tcopy(Wv[esl, :]),
                "bq": np.ascontiguousarray(bq[esl]).reshape(E, 1),
                "bk": np.ascontiguousarray(bk[esl]).reshape(E, 1),
                "bv": np.ascontiguousarray(bv[esl]).reshape(1, E),
                "maskT": maskT_all[b, h0 : h0 + HPC],
            }
        )
    return in_maps


_NC_CACHE = None
LAST_RESULT = None  # BassKernelResults of the most recent run (for profiling)


def kernel(**inputs) -> np.ndarray:
    global _NC_CACHE, LAST_RESULT
    _install_shims()
    from concourse.bass_utils import run_bass_kernel_spmd

    if _NC_CACHE is None:
        _NC_CACHE = build_nc()
    nc = _NC_CACHE

    in_maps = _prep_in_maps(**inputs)

    trace = bool(int(os.environ.get("KERNEL_TRACE", "0")))
    kwargs = {}
    if trace:
        kwargs["trace"] = True
        tmpdir = os.environ.get("KERNEL_TRACE_DIR")
        if tmpdir:
            kwargs["tmpdir"] = tmpdir
    res = run_bass_kernel_spmd(nc, in_maps, list(range(NCORES)), **kwargs)
    LAST_RESULT = res

    out = _alloc((B, N, D), np.float32)
    for c in range(NCORES):
        b = c // (NCORES // B)
        h0 = HPC * (c % (NCORES // B))
        aggT = res.results[c]["outT"]  # (E, N) unnormalized
        z = res.results[c]["zout"]  # (HPC, N)
        out[b, :, h0 * HD : (h0 + HPC) * HD] = (
            aggT / np.repeat(z, HD, axis=0)
        ).T
    return out



# revision 2
# speedup vs baseline: 1.9997x; 1.9997x over previous
"""MultiHeadCrossAttention kernel for 8 Trainium2 NeuronCores.

Reference computation (b=2, nq=nk=2048, d_model=512, h=8, hd=64):
    Q = split_heads(q @ Wq.T + bq); K, V likewise
    S = Q K^T * hd^-0.5 ; A = softmax(S, -1) * mask_head * diag(pearson)[k]
    out = merge_heads(A @ V)

Sharding: 16 (batch, head) pairs -> 2 heads of one batch per core.

Only the *diagonal* of pearson_matrix is used, so it is extracted on the
host and folded into the mask.  The QKV projections are tiny (O(N d^2))
next to the O(h N^2) attention term, so they run on the host (f32 BLAS)
and each core receives just its 2 heads' slices of Q^T/K^T/V in bf16.
The mask (the dominant memory term) is shipped in bf16 in a k-tile-major
layout so the device fetches it as 16 fully contiguous 1 MiB DMAs.

Device layout is "k on partitions, q on free axis":

    S^T[k,q]   = sum_d K^T[d,k] Q^T[d,q]     (TensorE, d=64, row-tiled 2 heads)
    E^T        = exp(SCALING * S^T)          (ScalarE, PSUM->SBUF bf16, 1024-wide)
    Z[q]      += ones^T E^T                  (TensorE, PSUM-accumulated over k)
    A^T        = E^T * maskT_folded          (VectorE, bf16 2x mode)
    agg^T[e,q]+= V[k,e]^T A^T[k,q]           (TensorE, PSUM-accumulated over k)
    out^T      = agg^T ; z                   (DVE copy -> DMA; host divides)

The device returns out^T (128 rows = 2 heads x 64 dims) and the softmax
denominators z; the host normalizes, transposes and concatenates.
"""

import ctypes
import os
import sys
import types

import numpy as np

import concourse.bacc as bacc
import concourse.bass as bass
import concourse.tile as tile
from concourse import mybir
from concourse.vector_clock import ScopedClock

F32 = mybir.dt.float32
BF16 = mybir.dt.bfloat16

B = 2
H = 8
N = 2048  # nq == nk
D = 512
HD = 64
HPC = 2  # heads per core
E = HPC * HD  # 128 output dims per core
SCALING = HD ** (-0.5)
NCORES = 8
P = 128
QC = 1024  # q super-chunk (2 per core)
NQC = N // QC
NKT = N // P  # 16 k tiles
HF = 512  # matmul free-dim chunk (one PSUM bank)


# ---------------------------------------------------------------------------
# Page faults are extremely slow in this sandbox (~ms each); MAP_POPULATE
# prefaults an allocation in one syscall, ~100x faster for big arrays.
# ---------------------------------------------------------------------------
_libc = ctypes.CDLL(None, use_errno=True)
_libc.mmap.restype = ctypes.c_void_p
_libc.mmap.argtypes = [
    ctypes.c_void_p,
    ctypes.c_size_t,
    ctypes.c_int,
    ctypes.c_int,
    ctypes.c_int,
    ctypes.c_long,
]


def _alloc(shape, dtype=np.float32):
    nbytes = int(np.prod(shape)) * np.dtype(dtype).itemsize
    nbytes = (nbytes + 4095) & ~4095
    p = _libc.mmap(None, nbytes, 0x3, 0x02 | 0x20 | 0x8000, -1, 0)  # RW, PRIV|ANON|POPULATE
    if p in (None, ctypes.c_void_p(-1).value):
        return np.empty(shape, dtype)
    buf = (ctypes.c_byte * nbytes).from_address(p)
    return np.frombuffer(buf, dtype=dtype, count=int(np.prod(shape))).reshape(shape)


# ---------------------------------------------------------------------------
# Environment shim: walrus in this container rejects >1 sync wait on
# CTRL-class instructions (NoOp/Drain), but TileContext's kernel-tail drain
# carries one wait per live semaphore.  Re-emit them as individual wait_ge
# instructions (one wait each) before a bare drain.
# ---------------------------------------------------------------------------
def _drain_and_barrier(self, tick_clock, wait_clock):
    probe = mybir.InstNoOp(
        name="wait_probe", ins=[], outs=[], engine=mybir.EngineType.SP
    )
    wait_clock.add_sem_waits(probe, ScopedClock({None: tick_clock.global_clock}))
    waits = list(probe.sync_info.on_wait) if probe.sync_info else []
    allocated = self.sems.allocated()
    by_name = {}
    for k, h in allocated.items():
        by_name[getattr(h, "name", str(k))] = h
    for w in waits:
        h = by_name.get(w.ant_name)
        assert h is not None, (w.ant_name, sorted(by_name))
        self.nc.sync.wait_ge(h, w.wait_value)
    self.nc.sync.drain()
    self.nc.all_engine_barrier()
    popped = self.nc._tile_sem_poison_stack.pop()
    assert popped is self._sem_poison
    self.nc.clear_and_free_semaphores(list(allocated.values()))
    self.nc.all_engine_barrier()


def _install_shims():
    tile.TileContext._drain_and_barrier = _drain_and_barrier
    if "antenv.axon_hooks" not in sys.modules:
        try:
            from trn_agent_boot.trn_boot import _ntff_profile_via_ctypes

            mod = types.ModuleType("antenv.axon_hooks")
            hook = _ntff_profile_via_ctypes("/opt/axon/libaxon_pjrt.so")
            mod.get_axon_ntff_profile_hook = lambda: hook
            mod.set_axon_ntff_profile_hook = lambda h: None
            sys.modules["antenv.axon_hooks"] = mod
        except Exception:
            pass


# ---------------------------------------------------------------------------
# Device kernel (one Bass program, SPMD over 8 cores; shards via in_maps)
# ---------------------------------------------------------------------------
def build_nc() -> bass.Bass:
    nc = bacc.Bacc("TRN2", target_bir_lowering=False, debug=False)

    qT = nc.dram_tensor("qT", [E, N], BF16, kind="ExternalInput")
    kT = nc.dram_tensor("kT", [E, N], BF16, kind="ExternalInput")
    vN = nc.dram_tensor("vN", [N, E], BF16, kind="ExternalInput")
    # maskR[kt, p, lh, q] = mask[b, h0+lh, q, kt*128+p] * diag(pearson)[b, h0+lh, kt*128+p]
    maskR = nc.dram_tensor("maskR", [NKT, P, HPC, N], BF16, kind="ExternalInput")
    outT = nc.dram_tensor("outT", [E, N], F32, kind="ExternalOutput")
    # softmax denominators, normalization happens on the host
    zout = nc.dram_tensor("zout", [HPC, N], F32, kind="ExternalOutput")

    with tile.TileContext(nc) as tc:
        with (
            tc.tile_pool(name="consts", bufs=1) as consts,
            tc.tile_pool(name="persist", bufs=1) as persist,
            tc.tile_pool(name="ps_s", bufs=2, space="PSUM") as ps_s,
            tc.tile_pool(name="ps_agg", bufs=1, space="PSUM") as ps_agg,
            tc.tile_pool(name="ps_z", bufs=1, space="PSUM") as ps_z,
            tc.tile_pool(name="et", bufs=3) as etp,
            tc.tile_pool(name="at", bufs=3) as atp,
            tc.tile_pool(name="outp", bufs=2) as outp,
            tc.tile_pool(name="zsb", bufs=2) as zsbp,
        ):
            ones = consts.tile([P, 1], BF16)
            nc.vector.memset(ones, 1.0)

            QT_sb = persist.tile([E, N], BF16, tag="QT")
            KT_sb = persist.tile([E, N], BF16, tag="KT")
            V_sb = persist.tile([P, NKT, E], BF16, tag="V")  # [k%128, kt, e]
            nc.sync.dma_start(out=QT_sb, in_=qT[:, :])
            nc.sync.dma_start(out=KT_sb, in_=kT[:, :])
            nc.sync.dma_start(out=V_sb, in_=vN[:, :].rearrange("(t p) e -> p t e", p=P))

            # Preload the whole (folded, bf16) mask: 16 x 1 MiB contiguous DMAs.
            mask_sb = []
            for kt in range(NKT):
                mt = persist.tile([P, HPC, N], BF16, tag=f"mask{kt}", name=f"mask{kt}")
                nc.sync.dma_start(out=mt, in_=maskR[kt])
                mask_sb.append(mt)

            for qc in range(NQC):
                qcols = slice(qc * QC, (qc + 1) * QC)
                agg = ps_agg.tile([P, QC], F32, tag="agg", name=f"agg_{qc}")
                zt = ps_z.tile([97, HF], F32, tag="zt", name=f"zt_{qc}")

                def emit_s(kt, lh):
                    """S^T matmuls for one (k-tile, head): [128k, 1024q] PSUM."""
                    ps = ps_s.tile([P, QC], F32, tag="s", name=f"s_{qc}_{kt}_{lh}")
                    lsl = slice(lh * HD, (lh + 1) * HD)
                    kcols = slice(kt * P, (kt + 1) * P)
                    for half in range(QC // HF):
                        rcols = slice(qc * QC + half * HF, qc * QC + (half + 1) * HF)
                        nc.tensor.matmul(
                            ps[:, half * HF : (half + 1) * HF],
                            KT_sb[lsl, kcols],
                            QT_sb[lsl, rcols],
                            start=True,
                            stop=True,
                            tile_position=(lh * HD, 0),
                        )
                    return ps

                def emit_rest(kt, lh, ps):
                    """exp + Z-accum + mask-mul + AV-accum for one (k-tile, head)."""
                    first, last = kt == 0, kt == NKT - 1
                    esl = slice(lh * HD, (lh + 1) * HD)
                    et = etp.tile([P, QC], BF16, tag="et", name=f"et_{qc}_{kt}_{lh}")
                    nc.scalar.activation(
                        et, ps, mybir.ActivationFunctionType.Exp, scale=SCALING
                    )
                    for half in range(QC // HF):
                        r = lh * HD + half * 32
                        nc.tensor.matmul(
                            zt[r : r + 1, :],
                            ones,
                            et[:, half * HF : (half + 1) * HF],
                            start=first,
                            stop=last,
                            tile_position=(0, r),
                            skip_group_check=True,
                        )
                    at = atp.tile([P, QC], BF16, tag="at", name=f"at_{qc}_{kt}_{lh}")
                    nc.vector.tensor_mul(at, et, mask_sb[kt][:, lh, qcols])
                    for half in range(QC // HF):
                        hsl = slice(half * HF, (half + 1) * HF)
                        nc.tensor.matmul(
                            agg[esl, hsl],
                            V_sb[:, kt, esl],
                            at[:, hsl],
                            start=first,
                            stop=last,
                            tile_position=(0, lh * HD),
                            skip_group_check=True,
                        )

                # Software pipeline: keep S one step ahead so TensorE always
                # has independent work while ScalarE/VectorE drain step s.
                steps = [(kt, lh) for kt in range(NKT) for lh in range(HPC)]
                prev = None
                for kt, lh in steps:
                    ps = emit_s(kt, lh)
                    if prev is not None:
                        emit_rest(*prev)
                    prev = (kt, lh, ps)
                emit_rest(*prev)

                # Epilogue: Z rows + agg out of PSUM, then DMA.
                zsb = zsbp.tile([97, HF], F32, tag="zsb", name=f"zsb_{qc}")
                for lh in range(HPC):
                    for half in range(QC // HF):
                        r = lh * HD + half * 32
                        nc.vector.tensor_copy(zsb[r : r + 1, :], zt[r : r + 1, :])
                        nc.sync.dma_start(
                            out=zout[lh, qc * QC + half * HF : qc * QC + (half + 1) * HF],
                            in_=zsb[r : r + 1, :],
                        )
                osb = outp.tile([P, QC], F32, tag="osb", name=f"osb_{qc}")
                nc.vector.tensor_copy(osb, agg)
                nc.sync.dma_start(out=outT[:, qcols], in_=osb)

    nc.compile()
    return nc


# ---------------------------------------------------------------------------
# Host side
# ---------------------------------------------------------------------------
def _prep_in_maps(q, k, v, mask_head, pearson_matrix, Wq, bq, Wk, bk, Wv, bv):
    import ml_dtypes

    f = np.float32
    bf = ml_dtypes.bfloat16
    q = np.asarray(q, f).reshape(B * N, D)
    k = np.asarray(k, f).reshape(B * N, D)
    v = np.asarray(v, f).reshape(B * N, D)
    mask_head = np.asarray(mask_head, f)
    Wq = np.asarray(Wq, f)
    Wk = np.asarray(Wk, f)
    Wv = np.asarray(Wv, f)
    bq = np.asarray(bq, f)
    bk = np.asarray(bk, f)
    bv = np.asarray(bv, f)

    # Host-side projections (f32 BLAS): tiny next to the O(h N^2) terms.
    Qf = (q @ Wq.T + bq).reshape(B, N, D)
    Kf = (k @ Wk.T + bk).reshape(B, N, D)
    Vf = (v @ Wv.T + bv).reshape(B, N, D)

    # Only the diagonal of pearson is used by the computation.
    pm = np.asarray(pearson_matrix)
    diag = np.ascontiguousarray(np.diagonal(pm, axis1=-2, axis2=-1)).astype(f)

    in_maps = []
    scratch = _alloc((N, N), f)  # f32 staging for one head's folded mask
    for c in range(NCORES):
        b = c // (NCORES // B)
        h0 = HPC * (c % (NCORES // B))
        esl = slice(h0 * HD, (h0 + HPC) * HD)

        qT_c = _alloc((E, N), bf)
        kT_c = _alloc((E, N), bf)
        vN_c = _alloc((N, E), bf)
        np.copyto(qT_c, Qf[b, :, esl].T)
        np.copyto(kT_c, Kf[b, :, esl].T)
        np.copyto(vN_c, Vf[b, :, esl])

        # maskR[kt, p, lh, q] = mask[b, h0+lh, q, kt*128+p] * diag[b, h0+lh, kt*128+p]
        maskR = _alloc((NKT, P, HPC, N), bf)
        for lh in range(HPC):
            h = h0 + lh
            np.multiply(mask_head[b, h].T, diag[b, h][:, None], out=scratch)
            np.copyto(maskR[:, :, lh, :], scratch.reshape(NKT, P, N))

        in_maps.append(
            {"qT": qT_c, "kT": kT_c, "vN": vN_c, "maskR": maskR}
        )
    return in_maps


_NC_CACHE = None
LAST_RESULT = None  # BassKernelResults of the most recent run (for profiling)


def kernel(**inputs) -> np.ndarray:
    global _NC_CACHE, LAST_RESULT
    _install_shims()
    from concourse.bass_utils import run_bass_kernel_spmd

    if _NC_CACHE is None:
        _NC_CACHE = build_nc()
    nc = _NC_CACHE

    in_maps = _prep_in_maps(**inputs)

    trace = bool(int(os.environ.get("KERNEL_TRACE", "0")))
    kwargs = {}
    if trace:
        kwargs["trace"] = True
        tmpdir = os.environ.get("KERNEL_TRACE_DIR")
        if tmpdir:
            kwargs["tmpdir"] = tmpdir
    res = run_bass_kernel_spmd(nc, in_maps, list(range(NCORES)), **kwargs)
    LAST_RESULT = res

    out = _alloc((B, N, D), np.float32)
    for c in range(NCORES):
        b = c // (NCORES // B)
        h0 = HPC * (c % (NCORES // B))
        aggT = res.results[c]["outT"]  # (E, N) unnormalized
        z = res.results[c]["zout"]  # (HPC, N)
        out[b, :, h0 * HD : (h0 + HPC) * HD] = (
            aggT / np.repeat(z, HD, axis=0)
        ).T
    return out


# revision 9
# speedup vs baseline: 2.0100x; 1.0052x over previous
"""MultiHeadCrossAttention kernel for 8 Trainium2 NeuronCores.

Reference computation (b=2, nq=nk=2048, d_model=512, h=8, hd=64):
    Q = split_heads(q @ Wq.T + bq); K, V likewise
    S = Q K^T * hd^-0.5 ; A = softmax(S, -1) * mask_head * diag(pearson)[k]
    out = merge_heads(A @ V)

Sharding: 16 (batch, head) pairs -> 2 heads of one batch per core.

Only the *diagonal* of pearson_matrix is used, so it is extracted on the
host and folded into the mask.  The QKV projections are tiny (O(N d^2))
next to the O(h N^2) attention term, so they run on the host (f32 BLAS)
and each core receives just its 2 heads' slices of Q^T/K^T/V in bf16.
The mask (the dominant memory term) is shipped in bf16 in a k-tile-major
layout so the device fetches it as 16 fully contiguous 1 MiB DMAs.

Device layout is "k on partitions, q on free axis":

    S^T[k,q]   = sum_d K^T[d,k] Q^T[d,q]     (TensorE, d=64, row-tiled 2 heads)
    E^T        = exp(SCALING * S^T)          (ScalarE, PSUM->SBUF bf16, 1024-wide)
    Z[q]      += ones^T E^T                  (TensorE, PSUM-accumulated over k)
    A^T        = E^T * maskT_folded          (VectorE, bf16 2x mode)
    agg^T[e,q]+= V[k,e]^T A^T[k,q]           (TensorE, PSUM-accumulated over k)
    out^T      = agg^T ; z                   (DVE copy -> DMA; host divides)

The device returns out^T (128 rows = 2 heads x 64 dims) and the softmax
denominators z; the host normalizes, transposes and concatenates.
"""

import ctypes
import os
import sys
import types

import numpy as np

import concourse.bacc as bacc
import concourse.bass as bass
import concourse.tile as tile
from concourse import mybir
from concourse.vector_clock import ScopedClock

F32 = mybir.dt.float32
BF16 = mybir.dt.bfloat16

B = 2
H = 8
N = 2048  # nq == nk
D = 512
HD = 64
HPC = 2  # heads per core
E = HPC * HD  # 128 output dims per core
SCALING = HD ** (-0.5)
NCORES = 8
P = 128
QC = 1024  # q super-chunk (2 per core)
NQC = N // QC
NKT = N // P  # 16 k tiles
HF = 512  # matmul free-dim chunk (one PSUM bank)


# ---------------------------------------------------------------------------
# Page faults are extremely slow in this sandbox (~ms each); MAP_POPULATE
# prefaults an allocation in one syscall, ~100x faster for big arrays.
# ---------------------------------------------------------------------------
_libc = ctypes.CDLL(None, use_errno=True)
_libc.mmap.restype = ctypes.c_void_p
_libc.mmap.argtypes = [
    ctypes.c_void_p,
    ctypes.c_size_t,
    ctypes.c_int,
    ctypes.c_int,
    ctypes.c_int,
    ctypes.c_long,
]


def _alloc(shape, dtype=np.float32):
    nbytes = int(np.prod(shape)) * np.dtype(dtype).itemsize
    nbytes = (nbytes + 4095) & ~4095
    p = _libc.mmap(None, nbytes, 0x3, 0x02 | 0x20 | 0x8000, -1, 0)  # RW, PRIV|ANON|POPULATE
    if p in (None, ctypes.c_void_p(-1).value):
        return np.empty(shape, dtype)
    buf = (ctypes.c_byte * nbytes).from_address(p)
    return np.frombuffer(buf, dtype=dtype, count=int(np.prod(shape))).reshape(shape)


# ---------------------------------------------------------------------------
# Environment shim: walrus in this container rejects >1 sync wait on
# CTRL-class instructions (NoOp/Drain), but TileContext's kernel-tail drain
# carries one wait per live semaphore.  Re-emit them as individual wait_ge
# instructions (one wait each) before a bare drain.
# ---------------------------------------------------------------------------
def _drain_and_barrier(self, tick_clock, wait_clock):
    probe = mybir.InstNoOp(
        name="wait_probe", ins=[], outs=[], engine=mybir.EngineType.SP
    )
    wait_clock.add_sem_waits(probe, ScopedClock({None: tick_clock.global_clock}))
    waits = list(probe.sync_info.on_wait) if probe.sync_info else []
    allocated = self.sems.allocated()
    by_name = {}
    for k, h in allocated.items():
        by_name[getattr(h, "name", str(k))] = h
    for w in waits:
        h = by_name.get(w.ant_name)
        assert h is not None, (w.ant_name, sorted(by_name))
        self.nc.sync.wait_ge(h, w.wait_value)
    self.nc.sync.drain()
    self.nc.all_engine_barrier()
    popped = self.nc._tile_sem_poison_stack.pop()
    assert popped is self._sem_poison
    self.nc.clear_and_free_semaphores(list(allocated.values()))
    self.nc.all_engine_barrier()


def _install_shims():
    tile.TileContext._drain_and_barrier = _drain_and_barrier
    if "antenv.axon_hooks" not in sys.modules:
        try:
            from trn_agent_boot.trn_boot import _ntff_profile_via_ctypes

            mod = types.ModuleType("antenv.axon_hooks")
            hook = _ntff_profile_via_ctypes("/opt/axon/libaxon_pjrt.so")
            mod.get_axon_ntff_profile_hook = lambda: hook
            mod.set_axon_ntff_profile_hook = lambda h: None
            sys.modules["antenv.axon_hooks"] = mod
        except Exception:
            pass


# ---------------------------------------------------------------------------
# Device kernel (one Bass program, SPMD over 8 cores; shards via in_maps)
# ---------------------------------------------------------------------------
def build_nc() -> bass.Bass:
    nc = bacc.Bacc("TRN2", target_bir_lowering=False, debug=False)

    qT = nc.dram_tensor("qT", [E, N], BF16, kind="ExternalInput")
    kT = nc.dram_tensor("kT", [E, N], BF16, kind="ExternalInput")
    vN = nc.dram_tensor("vN", [N, E], BF16, kind="ExternalInput")
    # maskR[kt, p, lh, q] = mask[b, h0+lh, q, kt*128+p] * diag(pearson)[b, h0+lh, kt*128+p]
    maskR = nc.dram_tensor("maskR", [NKT, P, HPC, N], BF16, kind="ExternalInput")
    outT = nc.dram_tensor("outT", [E, N], F32, kind="ExternalOutput")
    # softmax denominators as (lh*2+half, qc, i); normalization on the host
    zout = nc.dram_tensor("zout", [4, NQC, HF], F32, kind="ExternalOutput")

    with tile.TileContext(nc) as tc:
        with (
            tc.tile_pool(name="consts", bufs=1) as consts,
            tc.tile_pool(name="persist", bufs=1) as persist,
            tc.tile_pool(name="ps_s", bufs=2, space="PSUM") as ps_s,
            tc.tile_pool(name="ps_agg", bufs=1, space="PSUM") as ps_agg,
            tc.tile_pool(name="ps_z", bufs=1, space="PSUM") as ps_z,
            tc.tile_pool(name="ps_warm", bufs=1, space="PSUM") as ps_warm,
            tc.tile_pool(name="et", bufs=3) as etp,
            tc.tile_pool(name="at", bufs=3) as atp,
            tc.tile_pool(name="outp", bufs=2) as outp,
        ):
            ones = consts.tile([P, 1], BF16)
            nc.vector.memset(ones, 1.0)

            # PE warm-up: the HAM clock gate keeps the PE at 1.2 GHz until it
            # sees ~3.4us of sustained matmul activity.  Burn that in during
            # the DMA prefix (results discarded) so the real matmuls run at
            # 2.4 GHz from the first step.  Also pre-trigger the exp
            # table-load on ScalarE (~2.7us) with a dummy activation.
            warm_in = consts.tile([P, HF], BF16, tag="warm")
            nc.vector.memset(warm_in, 0.0)
            warm_act = consts.tile([P, 8], F32, tag="warmact")
            nc.scalar.activation(
                warm_act, warm_in[:, :8], mybir.ActivationFunctionType.Exp
            )
            wps = ps_warm.tile([P, HF], F32, tag="warmps")
            for i in range(10):
                nc.tensor.matmul(
                    wps, warm_in[:, :P], warm_in, start=True, stop=True
                )

            # Split Q^T/K^T/V loads so the first attention step only waits on
            # its own half (~0.75 MB) instead of the full 1.5 MB.
            QT_sb = persist.tile([E, N], BF16, tag="QT")
            KT_sb = persist.tile([E, N], BF16, tag="KT")
            V_sb = persist.tile([P, NKT, E], BF16, tag="V")  # [k%128, kt, e]
            mask_sb = [
                persist.tile([P, HPC, N], BF16, tag=f"mask{kt}", name=f"mask{kt}")
                for kt in range(NKT)
            ]
            vr = vN[:, :].rearrange("(t p) e -> p t e", p=P)
            HN = N // 2
            HT = NKT // 2
            nc.sync.dma_start(out=QT_sb[:, :HN], in_=qT[:, :HN])
            nc.sync.dma_start(out=KT_sb[:, :HN], in_=kT[:, :HN])
            nc.sync.dma_start(out=mask_sb[0], in_=maskR[0])
            nc.sync.dma_start(out=V_sb[:, :HT, :], in_=vr[:, :HT, :])
            nc.sync.dma_start(out=KT_sb[:, HN:], in_=kT[:, HN:])
            nc.sync.dma_start(out=V_sb[:, HT:, :], in_=vr[:, HT:, :])
            for kt in range(1, 5):
                nc.sync.dma_start(out=mask_sb[kt], in_=maskR[kt])
            nc.sync.dma_start(out=QT_sb[:, HN:], in_=qT[:, HN:])
            for kt in range(5, NKT):
                nc.sync.dma_start(out=mask_sb[kt], in_=maskR[kt])

            # z staging: rows {0,32,64,96} = (lh, half), free = (qc, q)
            zsb = persist.tile([97, NQC, HF], F32, tag="zsb")

            for qc in range(NQC):
                qcols = slice(qc * QC, (qc + 1) * QC)
                agg = ps_agg.tile([P, QC], F32, tag="agg", name=f"agg_{qc}")
                zt = ps_z.tile([97, HF], F32, tag="zt", name=f"zt_{qc}")

                def emit_s(kt, lh):
                    """S^T matmuls for one (k-tile, head): [128k, 1024q] PSUM."""
                    ps = ps_s.tile([P, QC], F32, tag="s", name=f"s_{qc}_{kt}_{lh}")
                    lsl = slice(lh * HD, (lh + 1) * HD)
                    kcols = slice(kt * P, (kt + 1) * P)
                    for half in range(QC // HF):
                        rcols = slice(qc * QC + half * HF, qc * QC + (half + 1) * HF)
                        nc.tensor.matmul(
                            ps[:, half * HF : (half + 1) * HF],
                            KT_sb[lsl, kcols],
                            QT_sb[lsl, rcols],
                            start=True,
                            stop=True,
                            tile_position=(lh * HD, 0),
                        )
                    return ps

                def emit_rest(kt, lh, ps):
                    """exp + Z-accum + mask-mul + AV-accum for one (k-tile, head)."""
                    first, last = kt == 0, kt == NKT - 1
                    esl = slice(lh * HD, (lh + 1) * HD)
                    et = etp.tile([P, QC], BF16, tag="et", name=f"et_{qc}_{kt}_{lh}")
                    nc.scalar.activation(
                        et, ps, mybir.ActivationFunctionType.Exp, scale=SCALING
                    )
                    for half in range(QC // HF):
                        r = lh * HD + half * 32
                        nc.tensor.matmul(
                            zt[r : r + 1, :],
                            ones,
                            et[:, half * HF : (half + 1) * HF],
                            start=first,
                            stop=last,
                            tile_position=(0, r),
                            skip_group_check=True,
                        )
                    at = atp.tile([P, QC], BF16, tag="at", name=f"at_{qc}_{kt}_{lh}")
                    nc.vector.tensor_mul(at, et, mask_sb[kt][:, lh, qcols])
                    for half in range(QC // HF):
                        hsl = slice(half * HF, (half + 1) * HF)
                        nc.tensor.matmul(
                            agg[esl, hsl],
                            V_sb[:, kt, esl],
                            at[:, hsl],
                            start=first,
                            stop=last,
                            tile_position=(0, lh * HD),
                            skip_group_check=True,
                        )

                # Software pipeline: keep S one step ahead so TensorE always
                # has independent work while ScalarE/VectorE drain step s.
                steps = [(kt, lh) for kt in range(NKT) for lh in range(HPC)]
                prev = None
                for kt, lh in steps:
                    ps = emit_s(kt, lh)
                    if prev is not None:
                        emit_rest(*prev)
                    prev = (kt, lh, ps)
                emit_rest(*prev)

                # Epilogue: agg + Z rows out of PSUM, then DMA.
                osb = outp.tile([P, QC], F32, tag="osb", name=f"osb_{qc}")
                nc.vector.tensor_copy(osb, agg)
                nc.sync.dma_start(out=outT[:, qcols], in_=osb)
                for lh in range(HPC):
                    for half in range(QC // HF):
                        r = lh * HD + half * 32
                        nc.vector.tensor_copy(zsb[r : r + 1, qc, :], zt[r : r + 1, :])

            # One coalesced zout DMA: SBUF rows {0,32,64,96} -> zout rows 0-3.
            nc.sync.dma_start(out=zout[:, :, :], in_=zsb[0:97:32, :, :])

    nc.compile()
    return nc


# ---------------------------------------------------------------------------
# Host side
# ---------------------------------------------------------------------------
def _prep_in_maps(q, k, v, mask_head, pearson_matrix, Wq, bq, Wk, bk, Wv, bv):
    import ml_dtypes

    f = np.float32
    bf = ml_dtypes.bfloat16
    q = np.asarray(q, f).reshape(B * N, D)
    k = np.asarray(k, f).reshape(B * N, D)
    v = np.asarray(v, f).reshape(B * N, D)
    mask_head = np.asarray(mask_head, f)
    Wq = np.asarray(Wq, f)
    Wk = np.asarray(Wk, f)
    Wv = np.asarray(Wv, f)
    bq = np.asarray(bq, f)
    bk = np.asarray(bk, f)
    bv = np.asarray(bv, f)

    # Host-side projections (f32 BLAS): tiny next to the O(h N^2) terms.
    Qf = (q @ Wq.T + bq).reshape(B, N, D)
    Kf = (k @ Wk.T + bk).reshape(B, N, D)
    Vf = (v @ Wv.T + bv).reshape(B, N, D)

    # Only the diagonal of pearson is used by the computation.
    pm = np.asarray(pearson_matrix)
    diag = np.ascontiguousarray(np.diagonal(pm, axis1=-2, axis2=-1)).astype(f)

    in_maps = []
    scratch = _alloc((N, N), f)  # f32 staging for one head's folded mask
    for c in range(NCORES):
        b = c // (NCORES // B)
        h0 = HPC * (c % (NCORES // B))
        esl = slice(h0 * HD, (h0 + HPC) * HD)

        qT_c = _alloc((E, N), bf)
        kT_c = _alloc((E, N), bf)
        vN_c = _alloc((N, E), bf)
        np.copyto(qT_c, Qf[b, :, esl].T)
        np.copyto(kT_c, Kf[b, :, esl].T)
        np.copyto(vN_c, Vf[b, :, esl])

        # maskR[kt, p, lh, q] = mask[b, h0+lh, q, kt*128+p] * diag[b, h0+lh, kt*128+p]
        maskR = _alloc((NKT, P, HPC, N), bf)
        for lh in range(HPC):
            h = h0 + lh
            np.multiply(mask_head[b, h].T, diag[b, h][:, None], out=scratch)
            np.copyto(maskR[:, :, lh, :], scratch.reshape(NKT, P, N))

        in_maps.append(
            {"qT": qT_c, "kT": kT_c, "vN": vN_c, "maskR": maskR}
        )
    return in_maps


_NC_CACHE = None
LAST_RESULT = None  # BassKernelResults of the most recent run (for profiling)


def kernel(**inputs) -> np.ndarray:
    global _NC_CACHE, LAST_RESULT
    _install_shims()
    from concourse.bass_utils import run_bass_kernel_spmd

    if _NC_CACHE is None:
        _NC_CACHE = build_nc()
    nc = _NC_CACHE

    in_maps = _prep_in_maps(**inputs)

    trace = bool(int(os.environ.get("KERNEL_TRACE", "0")))
    kwargs = {}
    if trace:
        kwargs["trace"] = True
        tmpdir = os.environ.get("KERNEL_TRACE_DIR")
        if tmpdir:
            kwargs["tmpdir"] = tmpdir
    res = run_bass_kernel_spmd(nc, in_maps, list(range(NCORES)), **kwargs)
    LAST_RESULT = res

    out = _alloc((B, N, D), np.float32)
    for c in range(NCORES):
        b = c // (NCORES // B)
        h0 = HPC * (c % (NCORES // B))
        aggT = res.results[c]["outT"]  # (E, N) unnormalized
        # zout rows are (lh*2+half, qc, i) -> z[lh, qc*QC + half*HF + i]
        zr = res.results[c]["zout"].reshape(HPC, 2, NQC, HF)
        z = zr.transpose(0, 2, 1, 3).reshape(HPC, N)
        out[b, :, h0 * HD : (h0 + HPC) * HD] = (
            aggT / np.repeat(z, HD, axis=0)
        ).T
    return out


# revision 13
# speedup vs baseline: 2.2564x; 1.1226x over previous
"""MultiHeadCrossAttention kernel for 8 Trainium2 NeuronCores.

Reference computation (b=2, nq=nk=2048, d_model=512, h=8, hd=64):
    Q = split_heads(q @ Wq.T + bq); K, V likewise
    S = Q K^T * hd^-0.5 ; A = softmax(S, -1) * mask_head * diag(pearson)[k]
    out = merge_heads(A @ V)

Sharding: 16 (batch, head) pairs -> 2 heads of one batch per core.

Only the *diagonal* of pearson_matrix is used, so it is extracted on the
host and folded into the mask.  The QKV projections are tiny (O(N d^2))
next to the O(h N^2) attention term, so they run on the host (f32 BLAS)
and each core receives just its 2 heads' slices of Q^T/K^T/V in bf16.
The mask (the dominant memory term) is shipped in bf16 in a k-tile-major
layout so the device fetches it as 16 fully contiguous 1 MiB DMAs.

Device layout is "k on partitions, q on free axis":

    S^T[k,q]   = sum_d K^T[d,k] Q^T[d,q]     (TensorE, d=64, row-tiled 2 heads)
    E^T        = exp(SCALING * S^T)          (ScalarE, PSUM->SBUF bf16, 1024-wide)
    Z[q]      += ones^T E^T                  (TensorE, PSUM-accumulated over k)
    A^T        = E^T * maskT_folded          (VectorE, bf16 2x mode)
    agg^T[e,q]+= V[k,e]^T A^T[k,q]           (TensorE, PSUM-accumulated over k)
    out^T      = agg^T ; z                   (DVE copy -> DMA; host divides)

The device returns out^T (128 rows = 2 heads x 64 dims) and the softmax
denominators z; the host normalizes, transposes and concatenates.
"""

import ctypes
import os
import sys
import types

import numpy as np

import concourse.bacc as bacc
import concourse.bass as bass
import concourse.tile as tile
from concourse import mybir
from concourse.vector_clock import ScopedClock

F32 = mybir.dt.float32
BF16 = mybir.dt.bfloat16

B = 2
H = 8
N = 2048  # nq == nk
D = 512
HD = 64
HPC = 2  # heads per core
E = HPC * HD  # 128 output dims per core
SCALING = HD ** (-0.5)
NCORES = 8
P = 128
QC = 1024  # q super-chunk (2 per core)
NQC = N // QC
NKT = N // P  # 16 k tiles
HF = 512  # matmul free-dim chunk (one PSUM bank)


# ---------------------------------------------------------------------------
# Page faults are extremely slow in this sandbox (~ms each); MAP_POPULATE
# prefaults an allocation in one syscall, ~100x faster for big arrays.
# ---------------------------------------------------------------------------
_libc = ctypes.CDLL(None, use_errno=True)
_libc.mmap.restype = ctypes.c_void_p
_libc.mmap.argtypes = [
    ctypes.c_void_p,
    ctypes.c_size_t,
    ctypes.c_int,
    ctypes.c_int,
    ctypes.c_int,
    ctypes.c_long,
]


def _alloc(shape, dtype=np.float32):
    nbytes = int(np.prod(shape)) * np.dtype(dtype).itemsize
    nbytes = (nbytes + 4095) & ~4095
    p = _libc.mmap(None, nbytes, 0x3, 0x02 | 0x20 | 0x8000, -1, 0)  # RW, PRIV|ANON|POPULATE
    if p in (None, ctypes.c_void_p(-1).value):
        return np.empty(shape, dtype)
    buf = (ctypes.c_byte * nbytes).from_address(p)
    return np.frombuffer(buf, dtype=dtype, count=int(np.prod(shape))).reshape(shape)


# ---------------------------------------------------------------------------
# Environment shim: walrus in this container rejects >1 sync wait on
# CTRL-class instructions (NoOp/Drain), but TileContext's kernel-tail drain
# carries one wait per live semaphore.  Re-emit them as individual wait_ge
# instructions (one wait each) before a bare drain.
# ---------------------------------------------------------------------------
def _drain_and_barrier(self, tick_clock, wait_clock):
    probe = mybir.InstNoOp(
        name="wait_probe", ins=[], outs=[], engine=mybir.EngineType.SP
    )
    wait_clock.add_sem_waits(probe, ScopedClock({None: tick_clock.global_clock}))
    waits = list(probe.sync_info.on_wait) if probe.sync_info else []
    allocated = self.sems.allocated()
    by_name = {}
    for k, h in allocated.items():
        by_name[getattr(h, "name", str(k))] = h
    for w in waits:
        h = by_name.get(w.ant_name)
        assert h is not None, (w.ant_name, sorted(by_name))
        self.nc.sync.wait_ge(h, w.wait_value)
    self.nc.sync.drain()
    self.nc.all_engine_barrier()
    popped = self.nc._tile_sem_poison_stack.pop()
    assert popped is self._sem_poison
    self.nc.clear_and_free_semaphores(list(allocated.values()))
    self.nc.all_engine_barrier()


def _install_shims():
    tile.TileContext._drain_and_barrier = _drain_and_barrier
    if "antenv.axon_hooks" not in sys.modules:
        try:
            from trn_agent_boot.trn_boot import _ntff_profile_via_ctypes

            mod = types.ModuleType("antenv.axon_hooks")
            hook = _ntff_profile_via_ctypes("/opt/axon/libaxon_pjrt.so")
            mod.get_axon_ntff_profile_hook = lambda: hook
            mod.set_axon_ntff_profile_hook = lambda h: None
            sys.modules["antenv.axon_hooks"] = mod
        except Exception:
            pass


# ---------------------------------------------------------------------------
# Device kernel (one Bass program, SPMD over 8 cores; shards via in_maps)
# ---------------------------------------------------------------------------
def build_nc() -> bass.Bass:
    nc = bacc.Bacc("TRN2", target_bir_lowering=False, debug=False)

    qT = nc.dram_tensor("qT", [E, N], BF16, kind="ExternalInput")
    kT = nc.dram_tensor("kT", [E, N], BF16, kind="ExternalInput")
    vN = nc.dram_tensor("vN", [N, E], BF16, kind="ExternalInput")
    # maskR[kt, p, lh, q] = mask[b, h0+lh, q, kt*128+p] * diag(pearson)[b, h0+lh, kt*128+p]
    maskR = nc.dram_tensor("maskR", [NKT, P, HPC, N], BF16, kind="ExternalInput")
    outT = nc.dram_tensor("outT", [E, N], F32, kind="ExternalOutput")
    # softmax denominators as (lh*2+half, qc, i); normalization on the host
    zout = nc.dram_tensor("zout", [4, NQC, HF], F32, kind="ExternalOutput")

    with tile.TileContext(nc) as tc:
        with (
            tc.tile_pool(name="consts", bufs=1) as consts,
            tc.tile_pool(name="persist", bufs=1) as persist,
            tc.tile_pool(name="ps_s", bufs=2, space="PSUM") as ps_s,
            tc.tile_pool(name="ps_agg", bufs=1, space="PSUM") as ps_agg,
            tc.tile_pool(name="ps_z", bufs=1, space="PSUM") as ps_z,
            tc.tile_pool(name="ps_warm", bufs=1, space="PSUM") as ps_warm,
            tc.tile_pool(name="et", bufs=3) as etp,
            tc.tile_pool(name="at", bufs=3) as atp,
            tc.tile_pool(name="outp", bufs=2) as outp,
        ):
            ones = consts.tile([P, 1], BF16)
            nc.vector.memset(ones, 1.0)

            # PE warm-up: the HAM clock gate keeps the PE at 1.2 GHz until it
            # sees ~3.4us of sustained matmul activity.  Burn that in during
            # the DMA prefix (results discarded) so the real matmuls run at
            # 2.4 GHz from the first step.  Also pre-trigger the exp
            # table-load on ScalarE (~2.7us) with a dummy activation.
            warm_in = consts.tile([P, HF], BF16, tag="warm")
            nc.vector.memset(warm_in, 0.0)
            warm_act = consts.tile([P, 8], F32, tag="warmact")
            nc.scalar.activation(
                warm_act, warm_in[:, :8], mybir.ActivationFunctionType.Exp
            )
            wps = ps_warm.tile([P, HF], F32, tag="warmps")
            for i in range(10):
                nc.tensor.matmul(
                    wps, warm_in[:, :P], warm_in, start=True, stop=True
                )

            # Split Q^T/K^T/V loads so the first attention step only waits on
            # its own half (~0.75 MB) instead of the full 1.5 MB.
            QT_sb = persist.tile([E, N], BF16, tag="QT")
            KT_sb = persist.tile([E, N], BF16, tag="KT")
            V_sb = persist.tile([P, NKT, E], BF16, tag="V")  # [k%128, kt, e]
            mask_sb = [
                persist.tile([P, HPC, N], BF16, tag=f"mask{kt}", name=f"mask{kt}")
                for kt in range(NKT)
            ]
            # Mask is fetched in q-halves: the qc=0 halves stream first (so
            # the first pass never waits ~1us per k-tile on full-row DMAs --
            # those stalls also kept the HAM clock-gate cold), the qc=1
            # halves follow during the first pass's compute.
            vr = vN[:, :].rearrange("(t p) e -> p t e", p=P)
            HN = N // 2
            HT = NKT // 2
            nc.sync.dma_start(out=QT_sb[:, :HN], in_=qT[:, :HN])
            nc.sync.dma_start(out=KT_sb[:, :HN], in_=kT[:, :HN])
            nc.sync.dma_start(out=mask_sb[0][:, :, :QC], in_=maskR[0][:, :, :QC])
            nc.sync.dma_start(out=V_sb[:, :HT, :], in_=vr[:, :HT, :])
            nc.sync.dma_start(out=mask_sb[1][:, :, :QC], in_=maskR[1][:, :, :QC])
            nc.sync.dma_start(out=KT_sb[:, HN:], in_=kT[:, HN:])
            nc.sync.dma_start(out=V_sb[:, HT:, :], in_=vr[:, HT:, :])
            for kt in range(2, NKT):
                nc.sync.dma_start(
                    out=mask_sb[kt][:, :, :QC], in_=maskR[kt][:, :, :QC]
                )
            nc.sync.dma_start(out=QT_sb[:, HN:], in_=qT[:, HN:])
            for kt in range(NKT):
                nc.sync.dma_start(
                    out=mask_sb[kt][:, :, QC:], in_=maskR[kt][:, :, QC:]
                )

            # z staging: rows {0,32,64,96} = (lh, half), free = (qc, q)
            zsb = persist.tile([97, NQC, HF], F32, tag="zsb")

            for qc in range(NQC):
                qcols = slice(qc * QC, (qc + 1) * QC)
                agg = ps_agg.tile([P, QC], F32, tag="agg", name=f"agg_{qc}")
                zt = ps_z.tile([97, HF], F32, tag="zt", name=f"zt_{qc}")

                def emit_s(kt, lh):
                    """S^T matmuls for one (k-tile, head): [128k, 1024q] PSUM."""
                    ps = ps_s.tile([P, QC], F32, tag="s", name=f"s_{qc}_{kt}_{lh}")
                    lsl = slice(lh * HD, (lh + 1) * HD)
                    kcols = slice(kt * P, (kt + 1) * P)
                    for half in range(QC // HF):
                        rcols = slice(qc * QC + half * HF, qc * QC + (half + 1) * HF)
                        nc.tensor.matmul(
                            ps[:, half * HF : (half + 1) * HF],
                            KT_sb[lsl, kcols],
                            QT_sb[lsl, rcols],
                            start=True,
                            stop=True,
                            tile_position=(lh * HD, 0),
                        )
                    return ps

                def emit_act(kt, lh, ps):
                    """exp for one (k-tile, head): PSUM f32 -> SBUF bf16."""
                    et = etp.tile([P, QC], BF16, tag="et", name=f"et_{qc}_{kt}_{lh}")
                    nc.scalar.activation(
                        et, ps, mybir.ActivationFunctionType.Exp, scale=SCALING
                    )
                    return et

                def emit_zav(kt, lh, et):
                    """Z-accum + mask-mul + AV-accum for one (k-tile, head).

                    Z rows are parked in the *other* head's PE column groups
                    so the Z and AV matmuls never share a 32-column group and
                    run concurrently.
                    """
                    first, last = kt == 0, kt == NKT - 1
                    esl = slice(lh * HD, (lh + 1) * HD)
                    for half in range(QC // HF):
                        r = (1 - lh) * HD + half * 32
                        nc.tensor.matmul(
                            zt[r : r + 1, :],
                            ones,
                            et[:, half * HF : (half + 1) * HF],
                            start=first,
                            stop=last,
                            tile_position=(0, r),
                            skip_group_check=True,
                        )
                    at = atp.tile([P, QC], BF16, tag="at", name=f"at_{qc}_{kt}_{lh}")
                    nc.vector.tensor_mul(at, et, mask_sb[kt][:, lh, qcols])
                    for half in range(QC // HF):
                        hsl = slice(half * HF, (half + 1) * HF)
                        nc.tensor.matmul(
                            agg[esl, hsl],
                            V_sb[:, kt, esl],
                            at[:, hsl],
                            start=first,
                            stop=last,
                            tile_position=(0, lh * HD),
                            skip_group_check=True,
                        )

                # Software pipeline, depth 2: S runs two steps ahead of Z/AV
                # so every matmul TensorE dequeues has its inputs long ready
                # -- the PE never stalls mid-queue waiting on exp/mask-mul.
                steps = [(kt, lh) for kt in range(NKT) for lh in range(HPC)]
                pipe = []
                for kt, lh in steps:
                    ps = emit_s(kt, lh)
                    if len(pipe) == 2:
                        emit_zav(*pipe.pop(0))
                    pipe.append((kt, lh, emit_act(kt, lh, ps)))
                while pipe:
                    emit_zav(*pipe.pop(0))

                # Epilogue: agg + Z rows out of PSUM, then DMA.
                osb = outp.tile([P, QC], F32, tag="osb", name=f"osb_{qc}")
                nc.vector.tensor_copy(osb, agg)
                nc.sync.dma_start(out=outT[:, qcols], in_=osb)
                for lh in range(HPC):
                    for half in range(QC // HF):
                        r = lh * HD + half * 32
                        nc.vector.tensor_copy(zsb[r : r + 1, qc, :], zt[r : r + 1, :])

            # One coalesced zout DMA: SBUF rows {0,32,64,96} -> zout rows 0-3.
            nc.sync.dma_start(out=zout[:, :, :], in_=zsb[0:97:32, :, :])

    nc.compile()
    return nc


# ---------------------------------------------------------------------------
# Host side
# ---------------------------------------------------------------------------
def _prep_in_maps(q, k, v, mask_head, pearson_matrix, Wq, bq, Wk, bk, Wv, bv):
    import ml_dtypes

    f = np.float32
    bf = ml_dtypes.bfloat16
    q = np.asarray(q, f).reshape(B * N, D)
    k = np.asarray(k, f).reshape(B * N, D)
    v = np.asarray(v, f).reshape(B * N, D)
    mask_head = np.asarray(mask_head, f)
    Wq = np.asarray(Wq, f)
    Wk = np.asarray(Wk, f)
    Wv = np.asarray(Wv, f)
    bq = np.asarray(bq, f)
    bk = np.asarray(bk, f)
    bv = np.asarray(bv, f)

    # Host-side projections (f32 BLAS): tiny next to the O(h N^2) terms.
    Qf = (q @ Wq.T + bq).reshape(B, N, D)
    Kf = (k @ Wk.T + bk).reshape(B, N, D)
    Vf = (v @ Wv.T + bv).reshape(B, N, D)

    # Only the diagonal of pearson is used by the computation.
    pm = np.asarray(pearson_matrix)
    diag = np.ascontiguousarray(np.diagonal(pm, axis1=-2, axis2=-1)).astype(f)

    in_maps = []
    scratch = _alloc((N, N), f)  # f32 staging for one head's folded mask
    for c in range(NCORES):
        b = c // (NCORES // B)
        h0 = HPC * (c % (NCORES // B))
        esl = slice(h0 * HD, (h0 + HPC) * HD)

        qT_c = _alloc((E, N), bf)
        kT_c = _alloc((E, N), bf)
        vN_c = _alloc((N, E), bf)
        np.copyto(qT_c, Qf[b, :, esl].T)
        np.copyto(kT_c, Kf[b, :, esl].T)
        np.copyto(vN_c, Vf[b, :, esl])

        # maskR[kt, p, lh, q] = mask[b, h0+lh, q, kt*128+p] * diag[b, h0+lh, kt*128+p]
        maskR = _alloc((NKT, P, HPC, N), bf)
        for lh in range(HPC):
            h = h0 + lh
            np.multiply(mask_head[b, h].T, diag[b, h][:, None], out=scratch)
            np.copyto(maskR[:, :, lh, :], scratch.reshape(NKT, P, N))

        in_maps.append(
            {"qT": qT_c, "kT": kT_c, "vN": vN_c, "maskR": maskR}
        )
    return in_maps


_NC_CACHE = None
LAST_RESULT = None  # BassKernelResults of the most recent run (for profiling)


def kernel(**inputs) -> np.ndarray:
    global _NC_CACHE, LAST_RESULT
    _install_shims()
    from concourse.bass_utils import run_bass_kernel_spmd

    if _NC_CACHE is None:
        _NC_CACHE = build_nc()
    nc = _NC_CACHE

    in_maps = _prep_in_maps(**inputs)

    trace = bool(int(os.environ.get("KERNEL_TRACE", "0")))
    kwargs = {}
    if trace:
        kwargs["trace"] = True
        tmpdir = os.environ.get("KERNEL_TRACE_DIR")
        if tmpdir:
            kwargs["tmpdir"] = tmpdir
    res = run_bass_kernel_spmd(nc, in_maps, list(range(NCORES)), **kwargs)
    LAST_RESULT = res

    out = _alloc((B, N, D), np.float32)
    for c in range(NCORES):
        b = c // (NCORES // B)
        h0 = HPC * (c % (NCORES // B))
        aggT = res.results[c]["outT"]  # (E, N) unnormalized
        # zout rows are ((1-lh)*2+half, qc, i) -> z[lh, qc*QC + half*HF + i]
        zr = res.results[c]["zout"].reshape(HPC, 2, NQC, HF)[::-1]
        z = zr.transpose(0, 2, 1, 3).reshape(HPC, N)
        out[b, :, h0 * HD : (h0 + HPC) * HD] = (
            aggT / np.repeat(z, HD, axis=0)
        ).T
    return out


# revision 18
# speedup vs baseline: 2.3319x; 1.0335x over previous
"""MultiHeadCrossAttention kernel for 8 Trainium2 NeuronCores.

Reference computation (b=2, nq=nk=2048, d_model=512, h=8, hd=64):
    Q = split_heads(q @ Wq.T + bq); K, V likewise
    S = Q K^T * hd^-0.5 ; A = softmax(S, -1) * mask_head * diag(pearson)[k]
    out = merge_heads(A @ V)

Sharding: 16 (batch, head) pairs -> 2 heads of one batch per core.

Only the *diagonal* of pearson_matrix is used, so it is extracted on the
host and folded into the mask.  The QKV projections are tiny (O(N d^2))
next to the O(h N^2) attention term, so they run on the host (f32 BLAS)
and each core receives just its 2 heads' slices of Q^T/K^T/V in bf16.
The mask (the dominant memory term) is shipped in bf16 in a k-tile-major
layout so the device fetches it as 16 fully contiguous 1 MiB DMAs.

Device layout is "k on partitions, q on free axis":

    S^T[k,q]   = sum_d K^T[d,k] Q^T[d,q]     (TensorE, d=64, row-tiled 2 heads)
    E^T        = exp(SCALING * S^T)          (ScalarE, PSUM->SBUF bf16, 1024-wide)
    Z[q]      += ones^T E^T                  (TensorE, PSUM-accumulated over k)
    A^T        = E^T * maskT_folded          (VectorE, bf16 2x mode)
    agg^T[e,q]+= V[k,e]^T A^T[k,q]           (TensorE, PSUM-accumulated over k)
    out^T      = agg^T ; z                   (DVE copy -> DMA; host divides)

The device returns out^T (128 rows = 2 heads x 64 dims) and the softmax
denominators z; the host normalizes, transposes and concatenates.
"""

import ctypes
import os
import sys
import types

import numpy as np

import concourse.bacc as bacc
import concourse.bass as bass
import concourse.tile as tile
from concourse import mybir
from concourse.vector_clock import ScopedClock

F32 = mybir.dt.float32
BF16 = mybir.dt.bfloat16

B = 2
H = 8
N = 2048  # nq == nk
D = 512
HD = 64
HPC = 2  # heads per core
E = HPC * HD  # 128 output dims per core
SCALING = HD ** (-0.5)
NCORES = 8
P = 128
QC = 1024  # q super-chunk (2 per core)
NQC = N // QC
NKT = N // P  # 16 k tiles
HF = 512  # matmul free-dim chunk (one PSUM bank)


# ---------------------------------------------------------------------------
# Page faults are extremely slow in this sandbox (~ms each); MAP_POPULATE
# prefaults an allocation in one syscall, ~100x faster for big arrays.
# ---------------------------------------------------------------------------
_libc = ctypes.CDLL(None, use_errno=True)
_libc.mmap.restype = ctypes.c_void_p
_libc.mmap.argtypes = [
    ctypes.c_void_p,
    ctypes.c_size_t,
    ctypes.c_int,
    ctypes.c_int,
    ctypes.c_int,
    ctypes.c_long,
]


def _alloc(shape, dtype=np.float32):
    nbytes = int(np.prod(shape)) * np.dtype(dtype).itemsize
    nbytes = (nbytes + 4095) & ~4095
    p = _libc.mmap(None, nbytes, 0x3, 0x02 | 0x20 | 0x8000, -1, 0)  # RW, PRIV|ANON|POPULATE
    if p in (None, ctypes.c_void_p(-1).value):
        return np.empty(shape, dtype)
    buf = (ctypes.c_byte * nbytes).from_address(p)
    return np.frombuffer(buf, dtype=dtype, count=int(np.prod(shape))).reshape(shape)


# ---------------------------------------------------------------------------
# Environment shim: walrus in this container rejects >1 sync wait on
# CTRL-class instructions (NoOp/Drain), but TileContext's kernel-tail drain
# carries one wait per live semaphore.  Re-emit them as individual wait_ge
# instructions (one wait each) before a bare drain.
# ---------------------------------------------------------------------------
def _drain_and_barrier(self, tick_clock, wait_clock):
    probe = mybir.InstNoOp(
        name="wait_probe", ins=[], outs=[], engine=mybir.EngineType.SP
    )
    wait_clock.add_sem_waits(probe, ScopedClock({None: tick_clock.global_clock}))
    waits = list(probe.sync_info.on_wait) if probe.sync_info else []
    allocated = self.sems.allocated()
    by_name = {}
    for k, h in allocated.items():
        by_name[getattr(h, "name", str(k))] = h
    for w in waits:
        h = by_name.get(w.ant_name)
        assert h is not None, (w.ant_name, sorted(by_name))
        self.nc.sync.wait_ge(h, w.wait_value)
    self.nc.sync.drain()
    self.nc.all_engine_barrier()
    popped = self.nc._tile_sem_poison_stack.pop()
    assert popped is self._sem_poison
    self.nc.clear_and_free_semaphores(list(allocated.values()))
    self.nc.all_engine_barrier()


def _install_shims():
    tile.TileContext._drain_and_barrier = _drain_and_barrier
    if "antenv.axon_hooks" not in sys.modules:
        try:
            from trn_agent_boot.trn_boot import _ntff_profile_via_ctypes

            mod = types.ModuleType("antenv.axon_hooks")
            hook = _ntff_profile_via_ctypes("/opt/axon/libaxon_pjrt.so")
            mod.get_axon_ntff_profile_hook = lambda: hook
            mod.set_axon_ntff_profile_hook = lambda h: None
            sys.modules["antenv.axon_hooks"] = mod
        except Exception:
            pass


# ---------------------------------------------------------------------------
# Device kernel (one Bass program, SPMD over 8 cores; shards via in_maps)
# ---------------------------------------------------------------------------
def build_nc() -> bass.Bass:
    nc = bacc.Bacc("TRN2", target_bir_lowering=False, debug=False)

    qT = nc.dram_tensor("qT", [E, N], BF16, kind="ExternalInput")
    kT = nc.dram_tensor("kT", [E, N], BF16, kind="ExternalInput")
    vN = nc.dram_tensor("vN", [N, E], BF16, kind="ExternalInput")
    # maskR[kt, p, lh, q] = mask[b, h0+lh, q, kt*128+p] * diag(pearson)[b, h0+lh, kt*128+p]
    maskR = nc.dram_tensor("maskR", [NKT, P, HPC, N], BF16, kind="ExternalInput")
    outT = nc.dram_tensor("outT", [E, N], F32, kind="ExternalOutput")
    # softmax denominators as (lh*2+half, qc, i); normalization on the host
    zout = nc.dram_tensor("zout", [4, NQC, HF], F32, kind="ExternalOutput")

    with tile.TileContext(nc) as tc:
        with (
            tc.tile_pool(name="consts", bufs=1) as consts,
            tc.tile_pool(name="persist", bufs=1) as persist,
            tc.tile_pool(name="ps_s", bufs=2, space="PSUM") as ps_s,
            tc.tile_pool(name="ps_agg", bufs=1, space="PSUM") as ps_agg,
            tc.tile_pool(name="ps_z", bufs=1, space="PSUM") as ps_z,
            tc.tile_pool(name="ps_warm", bufs=1, space="PSUM") as ps_warm,
            tc.tile_pool(name="et", bufs=4) as etp,
            tc.tile_pool(name="at", bufs=3) as atp,
            tc.tile_pool(name="outp", bufs=2) as outp,
        ):
            ones = consts.tile([P, 1], BF16)
            nc.vector.memset(ones, 1.0)

            # PE warm-up: the HAM clock gate keeps the PE at 1.2 GHz until it
            # sees ~3.4us of sustained matmul activity.  Burn that in during
            # the DMA prefix (results discarded) so the real matmuls run at
            # 2.4 GHz from the first step.  Also pre-trigger the exp
            # table-load on ScalarE (~2.7us) with a dummy activation.
            warm_in = consts.tile([P, HF], BF16, tag="warm")
            nc.vector.memset(warm_in, 0.0)
            warm_act = consts.tile([P, 8], F32, tag="warmact")
            nc.scalar.activation(
                warm_act, warm_in[:, :8], mybir.ActivationFunctionType.Exp
            )
            wps = ps_warm.tile([P, HF], F32, tag="warmps")
            for i in range(10):
                nc.tensor.matmul(
                    wps, warm_in[:, :P], warm_in, start=True, stop=True
                )

            # Split Q^T/K^T/V loads so the first attention step only waits on
            # its own half (~0.75 MB) instead of the full 1.5 MB.
            QT_sb = persist.tile([E, N], BF16, tag="QT")
            KT_sb = persist.tile([E, N], BF16, tag="KT")
            V_sb = persist.tile([P, NKT, E], BF16, tag="V")  # [k%128, kt, e]
            mask_sb = [
                persist.tile([P, HPC, N], BF16, tag=f"mask{kt}", name=f"mask{kt}")
                for kt in range(NKT)
            ]
            # Mask is fetched in q-halves: the qc=0 halves stream first (so
            # the first pass never waits ~1us per k-tile on full-row DMAs --
            # those stalls also kept the HAM clock-gate cold), the qc=1
            # halves follow during the first pass's compute.
            vr = vN[:, :].rearrange("(t p) e -> p t e", p=P)
            HN = N // 2
            HT = NKT // 2
            nc.sync.dma_start(out=KT_sb[:, :P], in_=kT[:, :P])
            nc.sync.dma_start(out=QT_sb[:, :HN], in_=qT[:, :HN])
            nc.sync.dma_start(out=KT_sb[:, P:HN], in_=kT[:, P:HN])
            nc.sync.dma_start(out=mask_sb[0][:, :, :QC], in_=maskR[0][:, :, :QC])
            nc.sync.dma_start(out=V_sb[:, :HT, :], in_=vr[:, :HT, :])
            nc.sync.dma_start(out=mask_sb[1][:, :, :QC], in_=maskR[1][:, :, :QC])
            nc.sync.dma_start(out=mask_sb[2][:, :, :QC], in_=maskR[2][:, :, :QC])
            nc.sync.dma_start(out=KT_sb[:, HN:], in_=kT[:, HN:])
            nc.sync.dma_start(out=mask_sb[3][:, :, :QC], in_=maskR[3][:, :, :QC])
            nc.sync.dma_start(out=mask_sb[4][:, :, :QC], in_=maskR[4][:, :, :QC])
            nc.sync.dma_start(out=V_sb[:, HT:, :], in_=vr[:, HT:, :])
            for kt in range(5, NKT):
                nc.sync.dma_start(
                    out=mask_sb[kt][:, :, :QC], in_=maskR[kt][:, :, :QC]
                )
            nc.sync.dma_start(out=QT_sb[:, HN:], in_=qT[:, HN:])
            for kt in range(NKT):
                nc.sync.dma_start(
                    out=mask_sb[kt][:, :, QC:], in_=maskR[kt][:, :, QC:]
                )

            # z staging: rows {0,32,64,96} = (lh, half), free = (qc, q)
            zsb = persist.tile([97, NQC, HF], F32, tag="zsb")

            for qc in range(NQC):
                qcols = slice(qc * QC, (qc + 1) * QC)
                agg = ps_agg.tile([P, QC], F32, tag="agg", name=f"agg_{qc}")
                zt = ps_z.tile([97, HF], F32, tag="zt", name=f"zt_{qc}")

                def emit_s(kt, lh):
                    """S^T matmuls for one (k-tile, head): [128k, 1024q] PSUM."""
                    ps = ps_s.tile([P, QC], F32, tag="s", name=f"s_{qc}_{kt}_{lh}")
                    lsl = slice(lh * HD, (lh + 1) * HD)
                    kcols = slice(kt * P, (kt + 1) * P)
                    for half in range(QC // HF):
                        rcols = slice(qc * QC + half * HF, qc * QC + (half + 1) * HF)
                        nc.tensor.matmul(
                            ps[:, half * HF : (half + 1) * HF],
                            KT_sb[lsl, kcols],
                            QT_sb[lsl, rcols],
                            start=True,
                            stop=True,
                            tile_position=(lh * HD, 0),
                        )
                    return ps

                def emit_act(kt, lh, ps):
                    """exp for one (k-tile, head): PSUM f32 -> SBUF bf16."""
                    et = etp.tile([P, QC], BF16, tag="et", name=f"et_{qc}_{kt}_{lh}")
                    nc.scalar.activation(
                        et, ps, mybir.ActivationFunctionType.Exp, scale=SCALING
                    )
                    return et

                def emit_z(kt, lh, et):
                    """Z-accum for one (k-tile, head).  Z rows are parked in
                    the *other* head's PE column groups; two steps' worth of
                    Z matmuls are emitted back-to-back (4 distinct 32-column
                    groups) so all four stream concurrently."""
                    first, last = kt == 0, kt == NKT - 1
                    for half in range(QC // HF):
                        r = (1 - lh) * HD + half * 32
                        nc.tensor.matmul(
                            zt[r : r + 1, :],
                            ones,
                            et[:, half * HF : (half + 1) * HF],
                            start=first,
                            stop=last,
                            tile_position=(0, r),
                            skip_group_check=True,
                        )

                def emit_av(kt, lh, et):
                    """mask-mul + AV-accum for one (k-tile, head)."""
                    first, last = kt == 0, kt == NKT - 1
                    esl = slice(lh * HD, (lh + 1) * HD)
                    at = atp.tile([P, QC], BF16, tag="at", name=f"at_{qc}_{kt}_{lh}")
                    nc.vector.tensor_mul(at, et, mask_sb[kt][:, lh, qcols])
                    for half in range(QC // HF):
                        hsl = slice(half * HF, (half + 1) * HF)
                        nc.tensor.matmul(
                            agg[esl, hsl],
                            V_sb[:, kt, esl],
                            at[:, hsl],
                            start=first,
                            stop=last,
                            tile_position=(0, lh * HD),
                            skip_group_check=True,
                        )

                # Software pipeline, depth 2: S runs two steps ahead of Z/AV
                # so every matmul TensorE dequeues has its inputs long ready
                # -- the PE never stalls mid-queue waiting on exp/mask-mul.
                # During the DMA-limited first steps, no-op PE fillers bridge
                # the mask-wait gaps so the HAM clock-gate sees a busy window
                # and unthrottles immediately (idle >1 window = re-throttle).
                steps = [(kt, lh) for kt in range(NKT) for lh in range(HPC)]
                pipe = []
                zq = []
                done = 0
                for idx, (kt, lh) in enumerate(steps):
                    ps = emit_s(kt, lh)
                    if qc == 0 and idx < 10:
                        nc.tensor.matmul(
                            wps, warm_in[:, :P], warm_in, start=True, stop=True
                        )
                    if len(pipe) == 2:
                        ent = pipe.pop(0)
                        zq.append(ent)
                        if len(zq) == 2 or ent[0] == NKT - 1:
                            for e in zq:
                                emit_z(*e)
                            zq = []
                        emit_av(*ent)
                    pipe.append((kt, lh, emit_act(kt, lh, ps)))
                while pipe:
                    ent = pipe.pop(0)
                    zq.append(ent)
                    if len(zq) == 2 or ent[0] == NKT - 1:
                        for e in zq:
                            emit_z(*e)
                        zq = []
                    emit_av(*ent)

                # Epilogue: Z rows + agg out of PSUM, agg in q-halves so the
                # out DMA starts earlier.
                for r in (0, 32, 64, 96):
                    nc.vector.tensor_copy(zsb[r : r + 1, qc, :], zt[r : r + 1, :])
                osb = outp.tile([P, QC], F32, tag="osb", name=f"osb_{qc}")
                for half in range(QC // HF):
                    hsl = slice(half * HF, (half + 1) * HF)
                    nc.vector.tensor_copy(osb[:, hsl], agg[:, hsl])
                    nc.sync.dma_start(
                        out=outT[:, qc * QC + half * HF : qc * QC + (half + 1) * HF],
                        in_=osb[:, hsl],
                    )

            # One coalesced zout DMA: SBUF rows {0,32,64,96} -> zout rows 0-3.
            nc.sync.dma_start(out=zout[:, :, :], in_=zsb[0:97:32, :, :])

    nc.compile()
    return nc


# ---------------------------------------------------------------------------
# Host side
# ---------------------------------------------------------------------------
def _prep_in_maps(q, k, v, mask_head, pearson_matrix, Wq, bq, Wk, bk, Wv, bv):
    import ml_dtypes

    f = np.float32
    bf = ml_dtypes.bfloat16
    q = np.asarray(q, f).reshape(B * N, D)
    k = np.asarray(k, f).reshape(B * N, D)
    v = np.asarray(v, f).reshape(B * N, D)
    mask_head = np.asarray(mask_head, f)
    Wq = np.asarray(Wq, f)
    Wk = np.asarray(Wk, f)
    Wv = np.asarray(Wv, f)
    bq = np.asarray(bq, f)
    bk = np.asarray(bk, f)
    bv = np.asarray(bv, f)

    # Host-side projections (f32 BLAS): tiny next to the O(h N^2) terms.
    Qf = (q @ Wq.T + bq).reshape(B, N, D)
    Kf = (k @ Wk.T + bk).reshape(B, N, D)
    Vf = (v @ Wv.T + bv).reshape(B, N, D)

    # Only the diagonal of pearson is used by the computation.
    pm = np.asarray(pearson_matrix)
    diag = np.ascontiguousarray(np.diagonal(pm, axis1=-2, axis2=-1)).astype(f)

    in_maps = []
    scratch = _alloc((N, N), f)  # f32 staging for one head's folded mask
    for c in range(NCORES):
        b = c // (NCORES // B)
        h0 = HPC * (c % (NCORES // B))
        esl = slice(h0 * HD, (h0 + HPC) * HD)

        qT_c = _alloc((E, N), bf)
        kT_c = _alloc((E, N), bf)
        vN_c = _alloc((N, E), bf)
        np.copyto(qT_c, Qf[b, :, esl].T)
        np.copyto(kT_c, Kf[b, :, esl].T)
        np.copyto(vN_c, Vf[b, :, esl])

        # maskR[kt, p, lh, q] = mask[b, h0+lh, q, kt*128+p] * diag[b, h0+lh, kt*128+p]
        maskR = _alloc((NKT, P, HPC, N), bf)
        for lh in range(HPC):
            h = h0 + lh
            np.multiply(mask_head[b, h].T, diag[b, h][:, None], out=scratch)
            np.copyto(maskR[:, :, lh, :], scratch.reshape(NKT, P, N))

        in_maps.append(
            {"qT": qT_c, "kT": kT_c, "vN": vN_c, "maskR": maskR}
        )
    return in_maps


_NC_CACHE = None
LAST_RESULT = None  # BassKernelResults of the most recent run (for profiling)


def kernel(**inputs) -> np.ndarray:
    global _NC_CACHE, LAST_RESULT
    _install_shims()
    from concourse.bass_utils import run_bass_kernel_spmd

    if _NC_CACHE is None:
        _NC_CACHE = build_nc()
    nc = _NC_CACHE

    in_maps = _prep_in_maps(**inputs)

    trace = bool(int(os.environ.get("KERNEL_TRACE", "0")))
    kwargs = {}
    if trace:
        kwargs["trace"] = True
        tmpdir = os.environ.get("KERNEL_TRACE_DIR")
        if tmpdir:
            kwargs["tmpdir"] = tmpdir
    res = run_bass_kernel_spmd(nc, in_maps, list(range(NCORES)), **kwargs)
    LAST_RESULT = res

    out = _alloc((B, N, D), np.float32)
    for c in range(NCORES):
        b = c // (NCORES // B)
        h0 = HPC * (c % (NCORES // B))
        aggT = res.results[c]["outT"]  # (E, N) unnormalized
        # zout rows are ((1-lh)*2+half, qc, i) -> z[lh, qc*QC + half*HF + i]
        zr = res.results[c]["zout"].reshape(HPC, 2, NQC, HF)[::-1]
        z = zr.transpose(0, 2, 1, 3).reshape(HPC, N)
        out[b, :, h0 * HD : (h0 + HPC) * HD] = (
            aggT / np.repeat(z, HD, axis=0)
        ).T
    return out


# revision 25
# speedup vs baseline: 2.7391x; 1.1746x over previous
"""MultiHeadCrossAttention kernel for 8 Trainium2 NeuronCores.

Reference computation (b=2, nq=nk=2048, d_model=512, h=8, hd=64):
    Q = split_heads(q @ Wq.T + bq); K, V likewise
    S = Q K^T * hd^-0.5 ; A = softmax(S, -1) * mask_head * diag(pearson)[k]
    out = merge_heads(A @ V)

Sharding: 16 (batch, head) pairs -> 2 heads of one batch per core.

Only the *diagonal* of pearson_matrix is used, so it is extracted on the
host and folded into the mask.  The QKV projections are tiny (O(N d^2))
next to the O(h N^2) attention term, so they run on the host (f32 BLAS)
and each core receives just its 2 heads' slices of Q^T/K^T/V in bf16.
The mask (the dominant memory term) is shipped in bf16 in a k-tile-major
layout so the device fetches it as 16 fully contiguous 1 MiB DMAs.

Device layout is "k on partitions, q on free axis":

    S^T[k,q]   = sum_d K^T[d,k] Q^T[d,q]     (TensorE, d=64, row-tiled 2 heads)
    E^T        = exp(SCALING * S^T)          (ScalarE, PSUM->SBUF bf16, 1024-wide)
    Z[q]      += ones^T E^T                  (TensorE, PSUM-accumulated over k)
    A^T        = E^T * maskT_folded          (VectorE, bf16 2x mode)
    agg^T[e,q]+= V[k,e]^T A^T[k,q]           (TensorE, PSUM-accumulated over k)
    out^T      = agg^T ; z                   (DVE copy -> DMA; host divides)

The device returns out^T (128 rows = 2 heads x 64 dims) and the softmax
denominators z; the host normalizes, transposes and concatenates.
"""

import ctypes
import os
import sys
import types

import numpy as np

import concourse.bacc as bacc
import concourse.bass as bass
import concourse.tile as tile
from concourse import mybir
from concourse.vector_clock import ScopedClock

F32 = mybir.dt.float32
BF16 = mybir.dt.bfloat16

B = 2
H = 8
N = 2048  # nq == nk
D = 512
HD = 64
HPC = 2  # heads per core
E = HPC * HD  # 128 output dims per core
SCALING = HD ** (-0.5)
NCORES = 8
P = 128
QC = 1024  # q super-chunk (2 per core)
NQC = N // QC
NKT = N // P  # 16 k tiles
HF = 512  # matmul free-dim chunk (one PSUM bank)


# ---------------------------------------------------------------------------
# Page faults are extremely slow in this sandbox (~ms each); MAP_POPULATE
# prefaults an allocation in one syscall, ~100x faster for big arrays.
# ---------------------------------------------------------------------------
_libc = ctypes.CDLL(None, use_errno=True)
_libc.mmap.restype = ctypes.c_void_p
_libc.mmap.argtypes = [
    ctypes.c_void_p,
    ctypes.c_size_t,
    ctypes.c_int,
    ctypes.c_int,
    ctypes.c_int,
    ctypes.c_long,
]


def _alloc(shape, dtype=np.float32):
    nbytes = int(np.prod(shape)) * np.dtype(dtype).itemsize
    nbytes = (nbytes + 4095) & ~4095
    p = _libc.mmap(None, nbytes, 0x3, 0x02 | 0x20 | 0x8000, -1, 0)  # RW, PRIV|ANON|POPULATE
    if p in (None, ctypes.c_void_p(-1).value):
        return np.empty(shape, dtype)
    buf = (ctypes.c_byte * nbytes).from_address(p)
    return np.frombuffer(buf, dtype=dtype, count=int(np.prod(shape))).reshape(shape)


# ---------------------------------------------------------------------------
# Environment shim: walrus in this container rejects >1 sync wait on
# CTRL-class instructions (NoOp/Drain), but TileContext's kernel-tail drain
# carries one wait per live semaphore.  Re-emit them as individual wait_ge
# instructions (one wait each) before a bare drain.
# ---------------------------------------------------------------------------
def _drain_and_barrier(self, tick_clock, wait_clock):
    probe = mybir.InstNoOp(
        name="wait_probe", ins=[], outs=[], engine=mybir.EngineType.SP
    )
    wait_clock.add_sem_waits(probe, ScopedClock({None: tick_clock.global_clock}))
    waits = list(probe.sync_info.on_wait) if probe.sync_info else []
    allocated = self.sems.allocated()
    by_name = {}
    for k, h in allocated.items():
        by_name[getattr(h, "name", str(k))] = h
    for w in waits:
        h = by_name.get(w.ant_name)
        assert h is not None, (w.ant_name, sorted(by_name))
        self.nc.sync.wait_ge(h, w.wait_value)
    self.nc.sync.drain()
    self.nc.all_engine_barrier()
    popped = self.nc._tile_sem_poison_stack.pop()
    assert popped is self._sem_poison
    self.nc.clear_and_free_semaphores(list(allocated.values()))
    self.nc.all_engine_barrier()


def _install_shims():
    tile.TileContext._drain_and_barrier = _drain_and_barrier
    if "antenv.axon_hooks" not in sys.modules:
        try:
            from trn_agent_boot.trn_boot import _ntff_profile_via_ctypes

            mod = types.ModuleType("antenv.axon_hooks")
            hook = _ntff_profile_via_ctypes("/opt/axon/libaxon_pjrt.so")
            mod.get_axon_ntff_profile_hook = lambda: hook
            mod.set_axon_ntff_profile_hook = lambda h: None
            sys.modules["antenv.axon_hooks"] = mod
        except Exception:
            pass


# ---------------------------------------------------------------------------
# Device kernel (one Bass program, SPMD over 8 cores; shards via in_maps)
# ---------------------------------------------------------------------------
def build_nc() -> bass.Bass:
    nc = bacc.Bacc("TRN2", target_bir_lowering=False, debug=False)

    qT = nc.dram_tensor("qT", [E, N], BF16, kind="ExternalInput")
    kT = nc.dram_tensor("kT", [E, N], BF16, kind="ExternalInput")
    vN = nc.dram_tensor("vN", [N, E], BF16, kind="ExternalInput")
    # maskR[kt, p, lh, q] = mask[b, h0+lh, q, kt*128+p] * diag(pearson)[b, h0+lh, kt*128+p]
    maskR = nc.dram_tensor("maskR", [NKT, P, HPC, N], BF16, kind="ExternalInput")
    outT = nc.dram_tensor("outT", [E, N], F32, kind="ExternalOutput")
    # softmax denominators as (lh*2+half, qc, i); normalization on the host
    zout = nc.dram_tensor("zout", [4, NQC, HF], F32, kind="ExternalOutput")

    with tile.TileContext(nc) as tc:
        with (
            tc.tile_pool(name="consts", bufs=1) as consts,
            tc.tile_pool(name="persist", bufs=1) as persist,
            tc.tile_pool(name="ps_s", bufs=2, space="PSUM") as ps_s,
            tc.tile_pool(name="ps_agg", bufs=1, space="PSUM") as ps_agg,
            tc.tile_pool(name="ps_z", bufs=1, space="PSUM") as ps_z,
            tc.tile_pool(name="ps_warm", bufs=1, space="PSUM") as ps_warm,
            tc.tile_pool(name="et", bufs=4) as etp,
            tc.tile_pool(name="at", bufs=4) as atp,
            tc.tile_pool(name="outp", bufs=2) as outp,
        ):
            ones = consts.tile([P, 1], BF16)
            nc.vector.memset(ones, 1.0)

            # PE warm-up: the HAM clock gate keeps the PE at 1.2 GHz until it
            # sees ~3.4us of sustained matmul activity.  Burn that in during
            # the DMA prefix (results discarded) so the real matmuls run at
            # 2.4 GHz from the first step.  Also pre-trigger the exp
            # table-load on ScalarE (~2.7us) with a dummy activation.
            warm_in = consts.tile([P, HF], BF16, tag="warm")
            nc.vector.memset(warm_in, 0.0)
            warm_act = consts.tile([P, 8], F32, tag="warmact")
            nc.scalar.activation(
                warm_act, warm_in[:, :8], mybir.ActivationFunctionType.Exp
            )
            wps = ps_warm.tile([P, HF], F32, tag="warmps")
            for i in range(10):
                nc.tensor.matmul(
                    wps, warm_in[:, :P], warm_in, start=True, stop=True
                )

            # Split Q^T/K^T/V loads so the first attention step only waits on
            # its own half (~0.75 MB) instead of the full 1.5 MB.
            QT_sb = persist.tile([E, N], BF16, tag="QT")
            KT_sb = persist.tile([E, N], BF16, tag="KT")
            V_sb = persist.tile([P, NKT, E], BF16, tag="V")  # [k%128, kt, e]
            mask_sb = [
                persist.tile([P, HPC, N], BF16, tag=f"mask{kt}", name=f"mask{kt}")
                for kt in range(NKT)
            ]
            # Mask is fetched in q-halves: the qc=0 halves stream first (so
            # the first pass never waits ~1us per k-tile on full-row DMAs --
            # those stalls also kept the HAM clock-gate cold), the qc=1
            # halves follow during the first pass's compute.
            vr = vN[:, :].rearrange("(t p) e -> p t e", p=P)
            HN = N // 2
            HT = NKT // 2
            # qkv rides the second HWDGE ring (qActDynamicHW, via the scalar
            # namespace) so the mask stream on the sync ring starts flowing
            # immediately -- the SDMA engines round-robin between rings.
            nc.scalar.dma_start(out=KT_sb[:, :P], in_=kT[:, :P])
            nc.scalar.dma_start(out=QT_sb[:, :HN], in_=qT[:, :HN])
            nc.scalar.dma_start(out=KT_sb[:, P:HN], in_=kT[:, P:HN])
            nc.scalar.dma_start(out=V_sb[:, :HT, :], in_=vr[:, :HT, :])
            nc.scalar.dma_start(out=KT_sb[:, HN:], in_=kT[:, HN:])
            nc.scalar.dma_start(out=V_sb[:, HT:, :], in_=vr[:, HT:, :])
            nc.scalar.dma_start(out=QT_sb[:, HN:], in_=qT[:, HN:])
            for kt in range(NKT):
                nc.sync.dma_start(
                    out=mask_sb[kt][:, :, :QC], in_=maskR[kt][:, :, :QC]
                )
            for kt in range(NKT):
                nc.sync.dma_start(
                    out=mask_sb[kt][:, :, QC:], in_=maskR[kt][:, :, QC:]
                )

            # z staging: rows {0,32,64,96} = (lh, half), free = (qc, q)
            zsb = persist.tile([97, NQC, HF], F32, tag="zsb")

            for qc in range(NQC):
                qcols = slice(qc * QC, (qc + 1) * QC)
                agg = ps_agg.tile([P, QC], F32, tag="agg", name=f"agg_{qc}")
                zt = ps_z.tile([97, HF], F32, tag="zt", name=f"zt_{qc}")

                def emit_s(kt, lh):
                    """S^T matmuls for one (k-tile, head): [128k, 1024q] PSUM."""
                    ps = ps_s.tile([P, QC], F32, tag="s", name=f"s_{qc}_{kt}_{lh}")
                    lsl = slice(lh * HD, (lh + 1) * HD)
                    kcols = slice(kt * P, (kt + 1) * P)
                    for half in range(QC // HF):
                        rcols = slice(qc * QC + half * HF, qc * QC + (half + 1) * HF)
                        nc.tensor.matmul(
                            ps[:, half * HF : (half + 1) * HF],
                            KT_sb[lsl, kcols],
                            QT_sb[lsl, rcols],
                            start=True,
                            stop=True,
                            tile_position=(lh * HD, 0),
                        )
                    return ps

                def emit_act(kt, lh, ps):
                    """exp for one (k-tile, head): PSUM f32 -> SBUF bf16."""
                    et = etp.tile([P, QC], BF16, tag="et", name=f"et_{qc}_{kt}_{lh}")
                    nc.scalar.activation(
                        et, ps, mybir.ActivationFunctionType.Exp, scale=SCALING
                    )
                    return et

                def emit_z(kt, lh, et):
                    """Z-accum for one (k-tile, head).  Z rows are parked in
                    the *other* head's PE column groups; two steps' worth of
                    Z matmuls are emitted back-to-back (4 distinct 32-column
                    groups) so all four stream concurrently."""
                    first, last = kt == 0, kt == NKT - 1
                    for half in range(QC // HF):
                        r = (1 - lh) * HD + half * 32
                        nc.tensor.matmul(
                            zt[r : r + 1, :],
                            ones,
                            et[:, half * HF : (half + 1) * HF],
                            start=first,
                            stop=last,
                            tile_position=(0, r),
                            skip_group_check=True,
                        )

                ats = {}

                def emit_mul(kt, lh, et):
                    """mask-mul for one (k-tile, head) on VectorE."""
                    at = atp.tile([P, QC], BF16, tag="at", name=f"at_{qc}_{kt}_{lh}")
                    nc.vector.tensor_mul(at, et, mask_sb[kt][:, lh, qcols])
                    ats[(kt, lh)] = at

                def emit_avmm(kt, lh, half):
                    """One AV-accum matmul; the flush interleaves these per
                    q-half across heads so consecutive matmuls use disjoint
                    PE column groups and overlap."""
                    first, last = kt == 0, kt == NKT - 1
                    esl = slice(lh * HD, (lh + 1) * HD)
                    hsl = slice(half * HF, (half + 1) * HF)
                    nc.tensor.matmul(
                        agg[esl, hsl],
                        V_sb[:, kt, esl],
                        ats[(kt, lh)][:, hsl],
                        start=first,
                        stop=last,
                        tile_position=(0, lh * HD),
                        skip_group_check=True,
                    )

                # Software pipeline, depth 2: S runs two steps ahead of Z/AV
                # so every matmul TensorE dequeues has its inputs long ready
                # -- the PE never stalls mid-queue waiting on exp/mask-mul.
                # During the DMA-limited first steps, no-op PE fillers bridge
                # the mask-wait gaps so the HAM clock-gate sees a busy window
                # and unthrottles immediately (idle >1 window = re-throttle).
                steps = [(kt, lh) for kt in range(NKT) for lh in range(HPC)]
                pipe = []
                batch = []

                def flush_batch():
                    # Two steps' worth (one full k-tile, both heads): the 4 Z
                    # matmuls stream concurrently (4 distinct column groups),
                    # then the 4 AV matmuls overlap pairwise (heads alternate
                    # column groups).
                    for e in batch:
                        emit_z(*e)
                    for e in batch:
                        emit_mul(*e)
                    for half in range(QC // HF):
                        for kt, lh, _ in batch:
                            emit_avmm(kt, lh, half)
                    for kt, lh, _ in batch:
                        del ats[(kt, lh)]
                    batch.clear()

                for idx, (kt, lh) in enumerate(steps):
                    ps = emit_s(kt, lh)
                    if qc == 0 and idx < 10:
                        nc.tensor.matmul(
                            wps, warm_in[:, :P], warm_in, start=True, stop=True
                        )
                    if len(pipe) == 2:
                        batch.append(pipe.pop(0))
                        if len(batch) == 2:
                            flush_batch()
                    pipe.append((kt, lh, emit_act(kt, lh, ps)))
                while pipe:
                    batch.append(pipe.pop(0))
                    if len(batch) == 2:
                        flush_batch()
                if batch:
                    flush_batch()

                # Epilogue: Z rows + agg out of PSUM, agg in q-halves so the
                # out DMA starts earlier.  On the last pass agg goes first
                # and the z copies run on the (now idle) scalar engine, off
                # the DVE critical path.
                last_qc = qc == NQC - 1
                if not last_qc:
                    for r in (0, 32, 64, 96):
                        nc.vector.tensor_copy(zsb[r : r + 1, qc, :], zt[r : r + 1, :])
                osb = outp.tile([P, QC], F32, tag="osb", name=f"osb_{qc}")
                for half in range(QC // HF):
                    hsl = slice(half * HF, (half + 1) * HF)
                    nc.vector.tensor_copy(osb[:, hsl], agg[:, hsl])
                    nc.sync.dma_start(
                        out=outT[:, qc * QC + half * HF : qc * QC + (half + 1) * HF],
                        in_=osb[:, hsl],
                    )
                if last_qc:
                    for r in (0, 32, 64, 96):
                        nc.scalar.copy(zsb[r : r + 1, qc, :], zt[r : r + 1, :])

            # One coalesced zout DMA: SBUF rows {0,32,64,96} -> zout rows 0-3.
            nc.sync.dma_start(out=zout[:, :, :], in_=zsb[0:97:32, :, :])

    nc.compile()
    return nc


# ---------------------------------------------------------------------------
# Host side
# ---------------------------------------------------------------------------
def _prep_in_maps(q, k, v, mask_head, pearson_matrix, Wq, bq, Wk, bk, Wv, bv):
    import ml_dtypes

    f = np.float32
    bf = ml_dtypes.bfloat16
    q = np.asarray(q, f).reshape(B * N, D)
    k = np.asarray(k, f).reshape(B * N, D)
    v = np.asarray(v, f).reshape(B * N, D)
    mask_head = np.asarray(mask_head, f)
    Wq = np.asarray(Wq, f)
    Wk = np.asarray(Wk, f)
    Wv = np.asarray(Wv, f)
    bq = np.asarray(bq, f)
    bk = np.asarray(bk, f)
    bv = np.asarray(bv, f)

    # Host-side projections (f32 BLAS): tiny next to the O(h N^2) terms.
    Qf = (q @ Wq.T + bq).reshape(B, N, D)
    Kf = (k @ Wk.T + bk).reshape(B, N, D)
    Vf = (v @ Wv.T + bv).reshape(B, N, D)

    # Only the diagonal of pearson is used by the computation.
    pm = np.asarray(pearson_matrix)
    diag = np.ascontiguousarray(np.diagonal(pm, axis1=-2, axis2=-1)).astype(f)

    in_maps = []
    scratch = _alloc((N, N), f)  # f32 staging for one head's folded mask
    for c in range(NCORES):
        b = c // (NCORES // B)
        h0 = HPC * (c % (NCORES // B))
        esl = slice(h0 * HD, (h0 + HPC) * HD)

        qT_c = _alloc((E, N), bf)
        kT_c = _alloc((E, N), bf)
        vN_c = _alloc((N, E), bf)
        np.copyto(qT_c, Qf[b, :, esl].T)
        np.copyto(kT_c, Kf[b, :, esl].T)
        np.copyto(vN_c, Vf[b, :, esl])

        # maskR[kt, p, lh, q] = mask[b, h0+lh, q, kt*128+p] * diag[b, h0+lh, kt*128+p]
        maskR = _alloc((NKT, P, HPC, N), bf)
        for lh in range(HPC):
            h = h0 + lh
            np.multiply(mask_head[b, h].T, diag[b, h][:, None], out=scratch)
            np.copyto(maskR[:, :, lh, :], scratch.reshape(NKT, P, N))

        in_maps.append(
            {"qT": qT_c, "kT": kT_c, "vN": vN_c, "maskR": maskR}
        )
    return in_maps


_NC_CACHE = None
LAST_RESULT = None  # BassKernelResults of the most recent run (for profiling)


def kernel(**inputs) -> np.ndarray:
    global _NC_CACHE, LAST_RESULT
    _install_shims()
    from concourse.bass_utils import run_bass_kernel_spmd

    if _NC_CACHE is None:
        _NC_CACHE = build_nc()
    nc = _NC_CACHE

    in_maps = _prep_in_maps(**inputs)

    trace = bool(int(os.environ.get("KERNEL_TRACE", "0")))
    kwargs = {}
    if trace:
        kwargs["trace"] = True
        tmpdir = os.environ.get("KERNEL_TRACE_DIR")
        if tmpdir:
            kwargs["tmpdir"] = tmpdir
    res = run_bass_kernel_spmd(nc, in_maps, list(range(NCORES)), **kwargs)
    LAST_RESULT = res

    out = _alloc((B, N, D), np.float32)
    for c in range(NCORES):
        b = c // (NCORES // B)
        h0 = HPC * (c % (NCORES // B))
        aggT = res.results[c]["outT"]  # (E, N) unnormalized
        # zout rows are ((1-lh)*2+half, qc, i) -> z[lh, qc*QC + half*HF + i]
        zr = res.results[c]["zout"].reshape(HPC, 2, NQC, HF)[::-1]
        z = zr.transpose(0, 2, 1, 3).reshape(HPC, N)
        out[b, :, h0 * HD : (h0 + HPC) * HD] = (
            aggT / np.repeat(z, HD, axis=0)
        ).T
    return out
